# revision 1
# baseline (speedup 1.0000x reference)
"""Trainium2 Bass kernel: GRU encoder-decoder with Bahdanau attention.

Model: B=4096, T=56 enc steps, S=28 dec steps, H=126.
Sharding: pure data parallel, batch 4096 -> 8 cores x 512.

Per-core layout:
  - GRU state h and all matmuls in [feature-on-partition, batch-on-free].
  - enc_out stored batch-partitioned [128b, 4c, 126h, 56t] bf16 for the
    attention weighted sum (reduce over innermost t on DVE).
  - Uo stored [126h, 56t, 512b] bf16; scores = V . tanh(Uo + Wh) via
    M=1 PE matmuls, evacuated PSUM->SBUF by DMA into batch-partitioned
    [128b, 4c, 56t] for the softmax over t.
  - sigmoid(x) computed as (1+tanh(x/2))/2 so the whole kernel uses one
    ACT table set (exp/tanh).
"""
import sys
import numpy as np

for _p in ('/opt/trn_rl_repo', '/root/.axon_site/_ro/trn_rl_repo'):
    if _p not in sys.path:
        sys.path.insert(0, _p)

from concourse import bass, tile
from concourse.vector_clock import ScopedClock

mybir = bass.mybir
F32 = mybir.dt.float32
BF16 = mybir.dt.bfloat16
AF = mybir.ActivationFunctionType
ALU = mybir.AluOpType
AX = mybir.AxisListType

# ---- workaround: this walrus build allows only one embedded sync-wait on
# the Tile tail drain; spread the global-clock waits over SP nops instead.
def _patched_drain_and_barrier(self, tick_clock, wait_clock):
    nc = self.nc
    probe = nc.sync.nop()
    wait_clock.add_sem_waits(probe.ins, ScopedClock({None: tick_clock.global_clock}))
    si = probe.ins.sync_info
    waits = list(si.on_wait or []) if si is not None else []
    if si is not None:
        si.on_wait = waits[:1]
    for w in waits[1:]:
        n2 = nc.sync.nop()
        n2.ins.sync_info = mybir.SyncInfo(on_wait=[w], on_update=[])
    nc.sync.drain()
    nc.all_engine_barrier()
    popped = nc._tile_sem_poison_stack.pop()
    assert popped is self._sem_poison
    nc.clear_and_free_semaphores(list(self.sems.allocated().values()))
    nc.all_engine_barrier()

tile.TileContext._drain_and_barrier = _patched_drain_and_barrier


def _split_excess_waits(nc):
    """This walrus build allows 1 embedded sync-wait per instruction; move
    extras onto same-engine nops inserted just before the instruction."""
    cnt = 0
    for _, bassbb in list(nc.bb_map.items()):
        bb = bassbb.bb if hasattr(bassbb, "bb") else bassbb
        il = bb.instructions
        i = 0
        while i < len(il):
            inst = il[i]
            si = inst.sync_info
            if si is not None and si.on_wait and len(si.on_wait) > 1:
                extra = list(si.on_wait[:-1])
                si.on_wait = [si.on_wait[-1]]
                for w in extra:
                    cnt += 1
                    nop = mybir.InstNoOp(name=f"wfix-{cnt}", ins=[], outs=[])
                    nop.engine = inst.engine
                    nop.sync_info = mybir.SyncInfo(on_wait=[w], on_update=[])
                    il.insert(i, nop)
                    i += 1
            i += 1
    return cnt

B, T, S = 4096, 56, 28
H, ANN, ENC, DEC = 126, 30, 20, 15
NCORES = 8
BS = B // NCORES          # 512 batch per core
NC4 = BS // 128           # 4 batch chunks of 128
TQ = 14                   # t-quarter for the attention product
TQS = 8                   # t-block for the scores pipeline
BBLK = 32                 # batch block per scores matmul

_CACHE = {}


def _build_program():
    import os
    kt = int(os.environ.get("K_T", T))
    ks = int(os.environ.get("K_S", S))
    klvl = int(os.environ.get("K_LVL", 9))
    nc = bass.Bass()

    di = lambda name, shape: nc.declare_dram_parameter(name, list(shape), F32, isOutput=False)
    enc_d = di("enc", (T, ENC, BS))
    dec_d = di("dec", (S, DEC, BS))
    ann_d = di("ann", (ANN, BS))
    W1_d = di("W1", (ANN, 96))
    W2_d = di("W2", (96, H))
    b1_d = di("b1", (96, 1))
    Wih_e_d = di("Wih_e", (ENC, 3 * H))
    Whh_e_d = di("Whh_e", (H, 3 * H))
    WihP_d_d = di("WihP_d", (1, 3 * H))
    WihX_d_d = di("WihX_d", (DEC, 3 * H))
    WihA_d_d = di("WihA_d", (H, 3 * H))
    Whh_d_d = di("Whh_d", (H, 3 * H))
    UW_d = di("UW", (H, H))
    WlW_d = di("WlW", (H, H))
    B_d = di("BIAS", (H, 16))
    h2ob_d = di("h2ob", (1, 1))
    id_d = di("ident", (128, 128))
    out_d = nc.declare_dram_parameter("out", [S, BS], F32, isOutput=True)

    from contextlib import ExitStack
    with tile.TileContext(nc) as tc, ExitStack() as es:
        cp = es.enter_context(tc.tile_pool(name="const", bufs=1))
        sp = es.enter_context(tc.tile_pool(name="sb", bufs=2))
        ppg = es.enter_context(tc.tile_pool(name="psg", bufs=5, space="PSUM"))
        ppsc = es.enter_context(tc.tile_pool(name="pssc", bufs=2, space="PSUM"))
        pptr = es.enter_context(tc.tile_pool(name="pstr", bufs=1, space="PSUM"))
        gp = es.enter_context(tc.tile_pool(name="gates", bufs=4))

        def cload(dram, shape, dtype=F32):
            t_ = cp.tile(list(shape), dtype, tag="c_" + dram.name)
            nc.sync.dma_start(out=t_[:], in_=dram[:])
            return t_

        W1 = cload(W1_d, (ANN, 96))
        W2 = cload(W2_d, (96, H))
        b1 = cload(b1_d, (96, 1))
        Wih_e = cload(Wih_e_d, (ENC, 3 * H))
        Whh_e = cload(Whh_e_d, (H, 3 * H))
        WihP = cload(WihP_d_d, (1, 3 * H))
        WihX = cload(WihX_d_d, (DEC, 3 * H))
        WihA = cload(WihA_d_d, (H, 3 * H))
        Whh_dd = cload(Whh_d_d, (H, 3 * H))
        UW = cload(UW_d, (H, H))
        WlW = cload(WlW_d, (H, H))
        BI = cload(B_d, (H, 16))
        h2ob = cload(h2ob_d, (1, 1))
        idf = cload(id_d, (128, 128))
        idb = cp.tile([128, 128], BF16, tag="idb")
        nc.vector.tensor_copy(idb[:], idf[:])
        Vb = cp.tile([H, 1], BF16, tag="Vb")
        nc.vector.tensor_copy(Vb[:], BI[:, 10:11])
        WihAb = cp.tile([H, 3 * H], BF16, tag="WihAb")
        nc.vector.tensor_copy(WihAb[:], WihA[:])

        # persistent big tensors
        Uo = cp.tile([H, T, BS], BF16, tag="Uo")          # 57.3 KB/part
        encb = cp.tile([128, NC4, H, T], BF16, tag="encb")  # 56.4 KB/part

        # bias columns
        bre, bze, bine, bhne = BI[:, 0:1], BI[:, 1:2], BI[:, 2:3], BI[:, 3:4]
        brd, bzd, bind, bhnd = BI[:, 4:5], BI[:, 5:6], BI[:, 6:7], BI[:, 7:8]
        Ub, Wlb, h2oW, b2 = BI[:, 8:9], BI[:, 9:10], BI[:, 11:12], BI[:, 12:13]

        # ---------------- phase A: static -> h0 ----------------
        annt = sp.tile([ANN, BS], F32, tag="x")
        nc.sync.dma_start(out=annt[:], in_=ann_d[:])
        ps96 = ppg.tile([96, BS], F32, tag="g")
        nc.tensor.matmul(ps96[:], W1[:], annt[:], start=True, stop=True)
        hid1 = gp.tile([96, BS], F32, tag="gate")
        nc.scalar.activation(hid1[:], ps96[:], AF.Relu, bias=b1[:, 0:1])
        psh = ppg.tile([H, BS], F32, tag="g")
        nc.tensor.matmul(psh[:], W2[:], hid1[:], start=True, stop=True)
        h_cur = sp.tile([H, BS], F32, tag="h")
        nc.scalar.activation(h_cur[:], psh[:], AF.Identity, bias=b2)

        # one GRU cell (encoder or decoder), consumes gate psums -> h_new
        def gru_tail(ps_r, ps_z, ps_ni, ps_hn, br_, bz_, bin_, bhn_, h_old):
            th_r = gp.tile([H, BS], F32, tag="gate")
            nc.scalar.activation(th_r[:], ps_r[:], AF.Tanh, bias=br_, scale=0.5)
            hnb = gp.tile([H, BS], F32, tag="gate")
            nc.scalar.activation(hnb[:], ps_hn[:], AF.Identity, bias=bhn_)
            tmp = gp.tile([H, BS], F32, tag="gate")
            nc.vector.scalar_tensor_tensor(tmp[:], th_r[:], 1.0, hnb[:], ALU.add, ALU.mult)
            pre = gp.tile([H, BS], F32, tag="gate")
            nc.vector.scalar_tensor_tensor(pre[:], tmp[:], 0.5, ps_ni[:], ALU.mult, ALU.add)
            n_ = gp.tile([H, BS], F32, tag="gate")
            nc.scalar.activation(n_[:], pre[:], AF.Tanh, bias=bin_)
            th_z = gp.tile([H, BS], F32, tag="gate")
            nc.scalar.activation(th_z[:], ps_z[:], AF.Tanh, bias=bz_, scale=0.5)
            d_ = gp.tile([H, BS], F32, tag="gate")
            nc.vector.tensor_sub(d_[:], n_[:], h_old[:])
            v1 = gp.tile([H, BS], F32, tag="gate")
            nc.vector.scalar_tensor_tensor(v1[:], th_z[:], -1.0, d_[:], ALU.add, ALU.mult)
            h_new = sp.tile([H, BS], F32, tag="h")
            nc.vector.scalar_tensor_tensor(h_new[:], v1[:], -0.5, h_old[:], ALU.mult, ALU.add)
            return h_new

        # ---------------- phase B: encoder ----------------
        for t in range(kt):
            xt = sp.tile([ENC, BS], F32, tag="x")
            nc.sync.dma_start(out=xt[:], in_=enc_d[t])
            ps_r = ppg.tile([H, BS], F32, tag="g")
            nc.tensor.matmul(ps_r[:], Wih_e[:, 0:H], xt[:], start=True, stop=False)
            nc.tensor.matmul(ps_r[:], Whh_e[:, 0:H], h_cur[:], start=False, stop=True)
            ps_z = ppg.tile([H, BS], F32, tag="g")
            nc.tensor.matmul(ps_z[:], Wih_e[:, H:2 * H], xt[:], start=True, stop=False)
            nc.tensor.matmul(ps_z[:], Whh_e[:, H:2 * H], h_cur[:], start=False, stop=True)
            ps_ni = ppg.tile([H, BS], F32, tag="g")
            nc.tensor.matmul(ps_ni[:], Wih_e[:, 2 * H:3 * H], xt[:], start=True, stop=True)
            ps_hn = ppg.tile([H, BS], F32, tag="g")
            nc.tensor.matmul(ps_hn[:], Whh_e[:, 2 * H:3 * H], h_cur[:], start=True, stop=True)
            h_new = gru_tail(ps_r, ps_z, ps_ni, ps_hn, bre, bze, bine, bhne, h_cur)

            # Uo[:, t, :] = U @ h_new + Ub   (bf16)
            ps_uo = ppg.tile([H, BS], F32, tag="g")
            nc.tensor.matmul(ps_uo[:], UW[:], h_new[:], start=True, stop=True)
            nc.scalar.activation(Uo[:, t, :], ps_uo[:], AF.Identity, bias=Ub)

            # enc_b[:, c, :, t] = h_new.T chunks (bf16)
            hb = sp.tile([H, BS], BF16, tag="hb")
            nc.vector.tensor_copy(hb[:], h_new[:])
            for c in range(NC4):
                ptr = pptr.tile([128, 128], BF16, tag="trb")
                nc.tensor.transpose(ptr[0:128, 0:H], hb[:, c * 128:(c + 1) * 128], idb[0:H, 0:H])
                nc.vector.tensor_copy(encb[:, c, :, t], ptr[0:128, 0:H])
            h_cur = h_new

        # ---------------- phase C: decoder ----------------
        prev = sp.tile([1, BS], F32, tag="pv")
        nc.sync.dma_start(out=prev[:], in_=enc_d[T - 1, 0:1, :])

        for s in range(ks):
            dxt = sp.tile([DEC, BS], F32, tag="dx")
            nc.sync.dma_start(out=dxt[:], in_=dec_d[s])

            # Wh = Wl @ h + Wlb  (bf16, shaped [H,1,BS] for t-broadcast)
            ps_wh = ppg.tile([H, BS], F32, tag="g")
            nc.tensor.matmul(ps_wh[:], WlW[:], h_cur[:], start=True, stop=True)
            wh = sp.tile([H, 1, BS], BF16, tag="wh")
            nc.scalar.activation(wh[:, 0, :], ps_wh[:], AF.Identity, bias=Wlb)

            scf = sp.tile([128, NC4, T], F32, tag="sco")
            for q in (range(T // TQS) if klvl >= 2 else []):
                xq = sp.tile([H, TQS, BS], BF16, tag="xt")
                nc.vector.tensor_add(xq[:], Uo[:, q * TQS:(q + 1) * TQS, :],
                                     wh[:].broadcast_to((H, TQS, BS)))
                nc.scalar.activation(xq[:], xq[:], AF.Tanh)
                xr = xq[:].rearrange("h t b -> h b t")
                for c in range(NC4):
                    pssc = ppsc.tile([128, BBLK, TQS], F32, tag="sc")
                    for j in range(4):
                        b0 = c * 128 + j * BBLK
                        nc.tensor.matmul(pssc[BBLK * j:BBLK * j + 1, :, :], Vb[:],
                                         xr[:, b0:b0 + BBLK, :], start=True, stop=True,
                                         tile_position=(0, BBLK * j))
                    sstg = sp.tile([128, BBLK, TQS], F32, tag="sst")
                    nc.scalar.copy(sstg[:], pssc[:])
                    nc.sync.dma_start(out=scf[:, c, q * TQS:(q + 1) * TQS],
                                      in_=sstg[0:128:BBLK])

            # softmax over t (free axis), batch-partitioned
            if klvl < 3:
                nc.vector.memset(scf[:], 0.125)
            mx = sp.tile([128, NC4], F32, tag="red")
            nc.vector.reduce_max(mx[:], scf[:], axis=AX.X)
            nmx = sp.tile([128, NC4], F32, tag="red")
            nc.vector.tensor_scalar_mul(nmx[:], mx[:], -1.0)
            expo = sp.tile([128, NC4, T], F32, tag="expo")
            for c in range(NC4):
                nc.scalar.activation(expo[:, c, :], scf[:, c, :], AF.Exp, bias=nmx[:, c:c + 1])
            sm = sp.tile([128, NC4], F32, tag="red")
            nc.vector.reduce_sum(sm[:], expo[:], axis=AX.X)
            inv = sp.tile([128, NC4], F32, tag="red")
            nc.vector.reciprocal(inv[:], sm[:])
            ab = sp.tile([128, NC4, 1, T], BF16, tag="ab")
            nc.vector.tensor_copy(ab[:, :, 0, :], expo[:])

            # attn = sum_t alpha * enc  (quarter-t passes, fold, reduce)
            Pacc = sp.tile([128, NC4, H, TQ], BF16, tag="P")
            Ptmp = sp.tile([128, NC4, H, TQ], BF16, tag="P")
            for q in range(T // TQ):
                es = encb[:, :, :, q * TQ:(q + 1) * TQ]
                asl = ab[:, :, :, q * TQ:(q + 1) * TQ].broadcast_to((128, NC4, H, TQ))
                if q == 0:
                    nc.vector.tensor_mul(Pacc[:], es, asl)
                else:
                    nc.vector.tensor_mul(Ptmp[:], es, asl)
                    nc.vector.tensor_add(Pacc[:], Pacc[:], Ptmp[:])
            nc.vector.tensor_add(Pacc[:, :, :, 0:7], Pacc[:, :, :, 0:7], Pacc[:, :, :, 7:14])
            araw = sp.tile([128, NC4, H], F32, tag="attn")
            nc.vector.tensor_reduce(araw[:], Pacc[:, :, :, 0:7], axis=AX.X, op=ALU.add)
            anrm = sp.tile([128, NC4, H], BF16, tag="attn")
            for c in range(NC4):
                nc.vector.tensor_scalar_mul(anrm[:, c, :], araw[:, c, :], inv[:, c:c + 1])

            # transpose attn back to [H, BS]
            attn_h = sp.tile([H, BS], BF16, tag="ah")
            for c in range(NC4):
                ptr = pptr.tile([128, 128], BF16, tag="trb")
                nc.tensor.transpose(ptr[0:H, 0:128], anrm[:, c, :], idb[:])
                nc.vector.tensor_copy(attn_h[:, c * 128:(c + 1) * 128], ptr[0:H, 0:128])

            # decoder GRU
            ps_r = ppg.tile([H, BS], F32, tag="g")
            nc.tensor.matmul(ps_r[:], WihP[:, 0:H], prev[:], start=True, stop=False)
            nc.tensor.matmul(ps_r[:], WihX[:, 0:H], dxt[:], start=False, stop=False)
            nc.tensor.matmul(ps_r[:], WihAb[:, 0:H], attn_h[:], start=False, stop=False)
            nc.tensor.matmul(ps_r[:], Whh_dd[:, 0:H], h_cur[:], start=False, stop=True)
            ps_z = ppg.tile([H, BS], F32, tag="g")
            nc.tensor.matmul(ps_z[:], WihP[:, H:2 * H], prev[:], start=True, stop=False)
            nc.tensor.matmul(ps_z[:], WihX[:, H:2 * H], dxt[:], start=False, stop=False)
            nc.tensor.matmul(ps_z[:], WihAb[:, H:2 * H], attn_h[:], start=False, stop=False)
            nc.tensor.matmul(ps_z[:], Whh_dd[:, H:2 * H], h_cur[:], start=False, stop=True)
            ps_ni = ppg.tile([H, BS], F32, tag="g")
            nc.tensor.matmul(ps_ni[:], WihP[:, 2 * H:3 * H], prev[:], start=True, stop=False)
            nc.tensor.matmul(ps_ni[:], WihX[:, 2 * H:3 * H], dxt[:], start=False, stop=False)
            nc.tensor.matmul(ps_ni[:], WihAb[:, 2 * H:3 * H], attn_h[:], start=False, stop=True)
            ps_hn = ppg.tile([H, BS], F32, tag="g")
            nc.tensor.matmul(ps_hn[:], Whh_dd[:, 2 * H:3 * H], h_cur[:], start=True, stop=True)
            h_new = gru_tail(ps_r, ps_z, ps_ni, ps_hn, brd, bzd, bind, bhnd, h_cur)

            # out_s = h2o @ h_new + b  -> DRAM, also feeds prev
            ps_o = ppsc.tile([1, BS], F32, tag="sc")
            nc.tensor.matmul(ps_o[:], h2oW, h_new[:], start=True, stop=True)
            prev = sp.tile([1, BS], F32, tag="pv")
            nc.scalar.activation(prev[:], ps_o[:], AF.Identity, bias=h2ob[:, 0:1])
            nc.sync.dma_start(out=out_d[s], in_=prev[:])
            h_cur = h_new
    _split_excess_waits(nc)
    return nc


def _host_inputs(inputs):
    f = lambda a: np.ascontiguousarray(a, dtype=np.float32)
    shared = {
        "W1": f(inputs["s2h_W1"].T), "W2": f(inputs["s2h_W2"].T),
        "b1": f(np.asarray(inputs["s2h_b1"]).reshape(96, 1)),
        "Wih_e": f(inputs["enc_Wih"].T), "Whh_e": f(inputs["enc_Whh"].T),
        "WihP_d": f(inputs["dec_Wih"][:, 0:1].T),
        "WihX_d": f(inputs["dec_Wih"][:, 1:1 + DEC].T),
        "WihA_d": f(inputs["dec_Wih"][:, 1 + DEC:].T),
        "Whh_d": f(inputs["dec_Whh"].T),
        "UW": f(inputs["U_W"].T), "WlW": f(inputs["Wl_W"].T),
        "h2ob": f(np.asarray(inputs["h2o_b"]).reshape(1, 1)),
        "ident": f(np.eye(128)),
    }
    BI = np.zeros((H, 16), dtype=np.float32)
    ebih, ebhh = np.asarray(inputs["enc_bih"]), np.asarray(inputs["enc_bhh"])
    dbih, dbhh = np.asarray(inputs["dec_bih"]), np.asarray(inputs["dec_bhh"])
    BI[:, 0] = (ebih[0:H] + ebhh[0:H]) / 2
    BI[:, 1] = (ebih[H:2 * H] + ebhh[H:2 * H]) / 2
    BI[:, 2] = ebih[2 * H:3 * H]
    BI[:, 3] = ebhh[2 * H:3 * H]
    BI[:, 4] = (dbih[0:H] + dbhh[0:H]) / 2
    BI[:, 5] = (dbih[H:2 * H] + dbhh[H:2 * H]) / 2
    BI[:, 6] = dbih[2 * H:3 * H]
    BI[:, 7] = dbhh[2 * H:3 * H]
    BI[:, 8] = np.asarray(inputs["U_b"])
    BI[:, 9] = np.asarray(inputs["Wl_b"])
    BI[:, 10] = np.asarray(inputs["V_W"]).reshape(-1)
    BI[:, 11] = np.asarray(inputs["h2o_W"]).reshape(-1)
    BI[:, 12] = np.asarray(inputs["s2h_b2"])
    shared["BIAS"] = f(BI)

    enc = np.asarray(inputs["encoder_data"])   # [T, B, ENC]
    dec = np.asarray(inputs["decoder_data"])   # [S, B, DEC]
    ann = np.asarray(inputs["ann_data"])       # [B, ANN]
    maps = []
    for i in range(NCORES):
        sl = slice(i * BS, (i + 1) * BS)
        m = dict(shared)
        m["enc"] = f(enc[:, sl, :].transpose(0, 2, 1))
        m["dec"] = f(dec[:, sl, :].transpose(0, 2, 1))
        m["ann"] = f(ann[sl, :].T)
        maps.append(m)
    return maps


def kernel(**inputs) -> np.ndarray:
    from concourse.bass_utils import run_bass_kernel_spmd
    if "nc" not in _CACHE:
        _CACHE["nc"] = _build_program()
    nc = _CACHE["nc"]
    maps = _host_inputs(inputs)
    import os
    kw = {}
    if os.environ.get("KERNEL_TRACE") == "1":
        kw = dict(trace=True, trace_cores=[0])
    res = run_bass_kernel_spmd(nc, maps, list(range(NCORES)), **kw)
    _CACHE["last_res"] = res
    outs = [np.asarray(res.results[i]["out"], dtype=np.float32) for i in range(NCORES)]
    full = np.stack(outs, axis=1)              # [S, 8, 512]
    return full.reshape(S, B, 1)


if __name__ == "__main__":
    rng = np.random.default_rng(0)
    fake = {}
    fake["ann_data"] = rng.standard_normal((B, ANN), dtype=np.float32)
    fake["encoder_data"] = rng.standard_normal((T, B, ENC), dtype=np.float32)
    fake["decoder_data"] = rng.standard_normal((S, B, DEC), dtype=np.float32)
    p = lambda *s: (rng.standard_normal(s, dtype=np.float32) * 0.08)
    fake.update({
        "s2h_W1": p(96, ANN), "s2h_b1": p(96), "s2h_W2": p(H, 96), "s2h_b2": p(H),
        "enc_Wih": p(3 * H, ENC), "enc_Whh": p(3 * H, H),
        "enc_bih": p(3 * H), "enc_bhh": p(3 * H),
        "dec_Wih": p(3 * H, 1 + DEC + H), "dec_Whh": p(3 * H, H),
        "dec_bih": p(3 * H), "dec_bhh": p(3 * H),
        "U_W": p(H, H), "U_b": p(H), "Wl_W": p(H, H), "Wl_b": p(H),
        "V_W": p(1, H), "V_b": p(1), "h2o_W": p(1, H), "h2o_b": p(1),
    })
    out = kernel(**fake)
    print("out", out.shape, out.dtype, float(np.abs(out).mean()))



# revision 13
# speedup vs baseline: 1.4219x; 1.4219x over previous
"""Trainium2 Bass kernel: GRU encoder-decoder with Bahdanau attention.

Model: B=4096, T=56 enc steps, S=28 dec steps, H=126.
Sharding: pure data parallel, batch 4096 -> 8 cores x 512.

v2 layout/engine plan:
  - All fp32 gate/proj matmuls run as float32r (single-pass PE, 1 cyc/row
    at N>=256) via AP bitcast; h state stays fp32.
  - GRU biases folded into ACT bias operands; the r*hn product uses
    pre-halved Whh_n plus a K=1 ones-row bias matmul so the hn PSUM is
    consumed directly by one scalar_tensor_tensor.
  - Decoder runs two batch halves (256 each) software-pipelined so the
    DVE/ACT serial chains of one half overlap the other's.
  - Scores: xq = tanh(Uo + Wh) as one DVE add + one ACT tanh per half;
    V-dot via M=1 matmuls (N=448, col-group packed 4x) -> PSUM -> direct
    DMA into batch-partitioned scf.
  - Softmax drops the max-subtraction (|score| <= ||V||_1 ~ 8, safe in
    fp32) and V_b (shift-invariant); 1/sum folded into alpha.
  - Attention sum: one TT mult + one bf16 tensor_reduce per half.
"""
import sys
import numpy as np

for _p in ('/opt/trn_rl_repo', '/root/.axon_site/_ro/trn_rl_repo'):
    if _p not in sys.path:
        sys.path.insert(0, _p)

from concourse import bass, tile
from concourse.vector_clock import ScopedClock

mybir = bass.mybir
F32 = mybir.dt.float32
F32R = mybir.dt.float32r
BF16 = mybir.dt.bfloat16
AF = mybir.ActivationFunctionType
ALU = mybir.AluOpType
AX = mybir.AxisListType

# ---- workaround: this walrus build allows only one embedded sync-wait on
# the Tile tail drain; spread the global-clock waits over SP nops instead.
def _patched_drain_and_barrier(self, tick_clock, wait_clock):
    nc = self.nc
    probe = nc.sync.nop()
    wait_clock.add_sem_waits(probe.ins, ScopedClock({None: tick_clock.global_clock}))
    si = probe.ins.sync_info
    waits = list(si.on_wait or []) if si is not None else []
    if si is not None:
        si.on_wait = waits[:1]
    for w in waits[1:]:
        n2 = nc.sync.nop()
        n2.ins.sync_info = mybir.SyncInfo(on_wait=[w], on_update=[])
    nc.sync.drain()
    nc.all_engine_barrier()
    popped = nc._tile_sem_poison_stack.pop()
    assert popped is self._sem_poison
    nc.clear_and_free_semaphores(list(self.sems.allocated().values()))
    nc.all_engine_barrier()

tile.TileContext._drain_and_barrier = _patched_drain_and_barrier


def _split_excess_waits(nc):
    """This walrus build allows 1 embedded sync-wait per instruction; move
    extras onto same-engine nops inserted just before the instruction."""
    cnt = 0
    for _, bassbb in list(nc.bb_map.items()):
        bb = bassbb.bb if hasattr(bassbb, "bb") else bassbb
        il = bb.instructions
        i = 0
        while i < len(il):
            inst = il[i]
            si = inst.sync_info
            if si is not None and si.on_wait and len(si.on_wait) > 1:
                extra = list(si.on_wait[:-1])
                si.on_wait = [si.on_wait[-1]]
                for w in extra:
                    cnt += 1
                    nop = mybir.InstNoOp(name=f"wfix-{cnt}", ins=[], outs=[])
                    nop.engine = inst.engine
                    nop.sync_info = mybir.SyncInfo(on_wait=[w], on_update=[])
                    il.insert(i, nop)
                    i += 1
            i += 1
    return cnt

B, T, S = 4096, 56, 28
H, ANN, ENC, DEC = 126, 30, 20, 15
NCORES = 8
BS = B // NCORES          # 512 batch per core
NH = 2                    # decoder batch halves (software pipeline)
HB = BS // NH             # 256 batch per half
NC2 = HB // 128           # 2 chunks of 128 per half
TQ = 14                   # t-block for the V-dot matmuls
BBLK = 32                 # batch block per V-dot matmul

_CACHE = {}


def _build_program():
    import os
    kt = int(os.environ.get("K_T", T))
    ks = int(os.environ.get("K_S", S))
    nc = bass.Bass()

    di = lambda name, shape, dt=F32: nc.declare_dram_parameter(name, list(shape), dt, isOutput=False)
    enc_d = di("enc", (T, ENC, BS), BF16)
    dec_d = di("dec", (S, DEC, BS), BF16)
    ann_d = di("ann", (ANN, BS), BF16)
    W1_d = di("W1", (ANN, 96), BF16)
    W2_d = di("W2", (96, H), BF16)
    b1_d = di("b1", (96, 1))
    Wih_e_d = di("Wih_e", (ENC, 3 * H), BF16)
    Whh_e_d = di("Whh_e", (H, 3 * H), BF16)
    WihP_d_d = di("WihP_d", (1, 3 * H), BF16)
    WihX_d_d = di("WihX_d", (DEC, 3 * H), BF16)
    WihA_d_d = di("WihA_d", (H, 3 * H), BF16)
    Whh_d_d = di("Whh_d", (H, 3 * H), BF16)
    UW_d = di("UW", (H, H), BF16)
    WlW_d = di("WlW", (H, H), BF16)
    B_d = di("BIAS", (H, 16))
    BR_d = di("BR", (1, 2 * H), BF16)
    h2ob_d = di("h2ob", (1, 1))
    id_d = di("ident", (128, 128))
    out_d = nc.declare_dram_parameter("out", [S, BS], BF16, isOutput=True)

    from contextlib import ExitStack
    with tile.TileContext(nc) as tc, ExitStack() as es:
        cp = es.enter_context(tc.tile_pool(name="const", bufs=1))
        sp = es.enter_context(tc.tile_pool(name="sb", bufs=2))
        hp = es.enter_context(tc.tile_pool(name="hs", bufs=4))
        xqp = es.enter_context(tc.tile_pool(name="xq", bufs=2))
        pbp = es.enter_context(tc.tile_pool(name="pb", bufs=2))
        ppg = es.enter_context(tc.tile_pool(name="psg", bufs=2, space="PSUM"))
        ppw = es.enter_context(tc.tile_pool(name="psw", bufs=1, space="PSUM"))
        ppsc = es.enter_context(tc.tile_pool(name="pssc", bufs=1, space="PSUM"))
        pptr = es.enter_context(tc.tile_pool(name="pstr", bufs=1, space="PSUM"))
        gp = es.enter_context(tc.tile_pool(name="gates", bufs=8))
        mp = es.enter_context(tc.tile_pool(name="misc", bufs=2))

        def cload(dram, shape, dtype=F32):
            t_ = cp.tile(list(shape), dtype, tag="c_" + dram.name)
            nc.sync.dma_start(out=t_[:], in_=dram[:])
            return t_

        W1 = cload(W1_d, (ANN, 96), BF16)
        W2 = cload(W2_d, (96, H), BF16)
        b1 = cload(b1_d, (96, 1))
        Wih_e = cload(Wih_e_d, (ENC, 3 * H), BF16)
        Whh_e = cload(Whh_e_d, (H, 3 * H), BF16)
        WihP = cload(WihP_d_d, (1, 3 * H), BF16)
        WihX = cload(WihX_d_d, (DEC, 3 * H), BF16)
        WihA = cload(WihA_d_d, (H, 3 * H), BF16)
        Whh_dd = cload(Whh_d_d, (H, 3 * H), BF16)
        UW = cload(UW_d, (H, H), BF16)
        WlW = cload(WlW_d, (H, H), BF16)
        BI = cload(B_d, (H, 16))
        BR = cload(BR_d, (1, 2 * H), BF16)
        h2ob = cload(h2ob_d, (1, 1))
        idf = cload(id_d, (128, 128))
        idb = cp.tile([128, 128], BF16, tag="idb")
        nc.vector.tensor_copy(idb[:], idf[:])
        Vb = cp.tile([H, 1], BF16, tag="Vb")
        nc.vector.tensor_copy(Vb[:], BI[:, 10:11])
        h2oWb = cp.tile([H, 1], BF16, tag="h2oWb")
        nc.vector.tensor_copy(h2oWb[:], BI[:, 11:12])
        ones = cp.tile([1, BS], BF16, tag="ones")
        nc.vector.memset(ones[:], 1.0)

        # persistent big tensors
        Uo = cp.tile([H, T, BS], BF16, tag="Uo")            # 57.3 KB/part
        encb = cp.tile([128, NH * NC2, H, T], BF16, tag="encb")  # 56.4 KB/part

        # bias columns (r/z biases pre-halved on host)
        bre, bze, bine = BI[:, 0:1], BI[:, 1:2], BI[:, 2:3]
        brd, bzd, bind = BI[:, 4:5], BI[:, 5:6], BI[:, 6:7]
        Ub, Wlb, h2oW, b2 = BI[:, 8:9], BI[:, 9:10], BI[:, 11:12], BI[:, 12:13]

        # ---------------- phase A: static -> h0 ----------------
        annt = sp.tile([ANN, BS], BF16, tag="x")
        nc.sync.dma_start(out=annt[:], in_=ann_d[:])
        ps96 = ppg.tile([96, BS], F32, tag="grz")
        nc.tensor.matmul(ps96[:], W1[:], annt[:], start=True, stop=True)
        hid1 = sp.tile([96, BS], BF16, tag="hid")
        nc.scalar.activation(hid1[:], ps96[:], AF.Relu, bias=b1[:, 0:1])
        psh = ppg.tile([H, BS], F32, tag="grz")
        nc.tensor.matmul(psh[:], W2[:], hid1[:], start=True, stop=True)
        hh = [hp.tile([H, HB], F32, tag=f"h{hf}", name=f"h{hf}") for hf in range(NH)]
        hbb = [hp.tile([H, HB], BF16, tag=f"hb{hf}", name=f"hb{hf}") for hf in range(NH)]
        for hf in range(NH):
            nc.scalar.activation(hh[hf][:], psh[:, hf * HB:(hf + 1) * HB],
                                 AF.Identity, bias=b2)
            nc.scalar.activation(hbb[hf][:], psh[:, hf * HB:(hf + 1) * HB],
                                 AF.Identity, bias=b2)

        # one GRU tail, consumes gate psums -> h_new (per half)
        # ps_rz: [H,2,HB] (r,z); ps_nh: [H,2,HB] (ni, hn_half incl bias)
        def gru_tail(ps_rz, ps_nh, br_, bz_, bin_, h_old, hf):
            th_r = gp.tile([H, HB], F32, tag="gate")
            nc.scalar.activation(th_r[:], ps_rz[:, 0, :], AF.Tanh, bias=br_, scale=0.5)
            th_z = gp.tile([H, HB], F32, tag="gate")
            nc.scalar.activation(th_z[:], ps_rz[:, 1, :], AF.Tanh, bias=bz_, scale=0.5)
            tmp = gp.tile([H, HB], F32, tag="gate")
            nc.vector.scalar_tensor_tensor(tmp[:], th_r[:], 1.0, ps_nh[:, 1, :],
                                           ALU.add, ALU.mult)
            pre = gp.tile([H, HB], F32, tag="gate")
            nc.vector.tensor_add(pre[:], tmp[:], ps_nh[:, 0, :])
            n_ = gp.tile([H, HB], F32, tag="gate")
            nc.scalar.activation(n_[:], pre[:], AF.Tanh, bias=bin_)
            d_ = gp.tile([H, HB], F32, tag="gate")
            nc.vector.tensor_sub(d_[:], n_[:], h_old[:])
            v1 = gp.tile([H, HB], F32, tag="gate")
            nc.vector.scalar_tensor_tensor(v1[:], th_z[:], -1.0, d_[:], ALU.add, ALU.mult)
            h_new = hp.tile([H, HB], F32, tag=f"h{hf}")
            nc.vector.scalar_tensor_tensor(h_new[:], v1[:], -0.5, h_old[:],
                                           ALU.mult, ALU.add)
            hb_new = hp.tile([H, HB], BF16, tag=f"hb{hf}", name=f"hbn{hf}")
            nc.vector.tensor_copy(hb_new[:], h_new[:])
            return h_new, hb_new

        # ---------------- phase B: encoder ----------------
        for t in range(kt):
            xt = sp.tile([ENC, BS], BF16, tag="x")
            nc.sync.dma_start(out=xt[:], in_=enc_d[t])
            for hf in range(NH):
                sl = slice(hf * HB, (hf + 1) * HB)
                h_old, hb_old = hh[hf], hbb[hf]
                ps_rz = ppg.tile([H, 2, HB], F32, tag="grz")
                nc.tensor.matmul(ps_rz[:, 0, :], Wih_e[:, 0:H], xt[:, sl],
                                 start=True, stop=False)
                nc.tensor.matmul(ps_rz[:, 0, :], Whh_e[:, 0:H], hb_old[:],
                                 start=False, stop=True)
                nc.tensor.matmul(ps_rz[:, 1, :], Wih_e[:, H:2 * H], xt[:, sl],
                                 start=True, stop=False)
                nc.tensor.matmul(ps_rz[:, 1, :], Whh_e[:, H:2 * H], hb_old[:],
                                 start=False, stop=True)
                ps_nh = ppg.tile([H, 2, HB], F32, tag="gnh")
                nc.tensor.matmul(ps_nh[:, 0, :], Wih_e[:, 2 * H:3 * H], xt[:, sl],
                                 start=True, stop=True)
                nc.tensor.matmul(ps_nh[:, 1, :], Whh_e[:, 2 * H:3 * H], hb_old[:],
                                 start=True, stop=False)
                nc.tensor.matmul(ps_nh[:, 1, :], BR[0:1, 0:H], ones[:, sl],
                                 start=False, stop=True)
                h_new, hb_new = gru_tail(ps_rz, ps_nh, bre, bze, bine, h_old, hf)

                # Uo[:, t, sl] = U @ h_new + Ub   (bf16)
                ps_uo = ppw.tile([H, HB], F32, tag="wh")
                nc.tensor.matmul(ps_uo[:], UW[:], hb_new[:], start=True, stop=True)
                nc.scalar.activation(Uo[:, t, sl], ps_uo[:], AF.Identity, bias=Ub)

                # encb[:, 2hf+c2, :, t] = h_new.T chunks (bf16)
                for c2 in range(NC2):
                    ptr = pptr.tile([128, 128], BF16, tag="trb")
                    nc.tensor.transpose(ptr[0:128, 0:H],
                                        hb_new[:, c2 * 128:(c2 + 1) * 128],
                                        idb[0:H, 0:H])
                    nc.scalar.copy(encb[:, NC2 * hf + c2, :, t], ptr[0:128, 0:H])
                hh[hf], hbb[hf] = h_new, hb_new

        # ---------------- phase C: decoder ----------------
        prevs = []
        for hf in range(NH):
            pv = hp.tile([1, HB], BF16, tag=f"pv{hf}")
            nc.sync.dma_start(out=pv[:], in_=enc_d[T - 1, 0:1, hf * HB:(hf + 1) * HB])
            prevs.append(pv)

        for s in range(ks):
            dxt = sp.tile([DEC, BS], BF16, tag="dx")
            nc.sync.dma_start(out=dxt[:], in_=dec_d[s])

            scfs, attns = {}, {}
            # stage W + scores pipeline (t-chunks of TQ) per half
            for hf in range(NH):
                sl = slice(hf * HB, (hf + 1) * HB)
                ps_wh = ppw.tile([H, HB], F32, tag="wh")
                nc.tensor.matmul(ps_wh[:], WlW[:], hbb[hf][:], start=True, stop=True)
                wh = mp.tile([H, 1, HB], BF16, tag="whb")
                nc.scalar.activation(wh[:, 0, :], ps_wh[:], AF.Identity, bias=Wlb)
                scf = sp.tile([128, NC2, T], F32, tag="sco")
                for q in range(T // TQ):
                    tq = slice(q * TQ, (q + 1) * TQ)
                    xq = xqp.tile([H, TQ, HB], BF16, tag="xt")
                    nc.vector.tensor_add(xq[:], Uo[:, tq, sl],
                                         wh[:].broadcast_to((H, TQ, HB)))
                    nc.scalar.activation(xq[:], xq[:], AF.Tanh)
                    xr = xq[:].rearrange("h t b -> h b t")
                    for c2 in range(NC2):
                        pssc = ppsc.tile([128, BBLK, TQ], F32, tag="sc")
                        for j in range(4):
                            b0 = c2 * 128 + j * BBLK
                            nc.tensor.matmul(pssc[BBLK * j:BBLK * j + 1, :, :], Vb[:],
                                             xr[:, b0:b0 + BBLK, :],
                                             start=True, stop=True,
                                             tile_position=(0, BBLK * j))
                        sstg = mp.tile([128, BBLK, TQ], F32, tag="sst")
                        nc.scalar.copy(sstg[:], pssc[:])
                        nc.sync.dma_start(out=scf[:, c2, tq],
                                          in_=sstg[0:128:BBLK])
                scfs[hf] = scf

            # stage softmax (no max-subtraction; scores bounded by ||V||_1)
            # + attention weighted sum + transpose back
            for hf in range(NH):
                scf = scfs[hf]
                expo = sp.tile([128, NC2, T], F32, tag="expo")
                nc.scalar.activation(expo[:], scf[:], AF.Exp)
                sm = sp.tile([128, NC2], F32, tag="red")
                nc.vector.tensor_reduce(sm[:], expo[:], axis=AX.X, op=ALU.add)
                inv = sp.tile([128, NC2], F32, tag="red2")
                nc.vector.reciprocal(inv[:], sm[:])
                ab = sp.tile([128, NC2, 1, T], BF16, tag="ab")
                for c2 in range(NC2):
                    nc.vector.tensor_scalar_mul(ab[:, c2, 0, :], expo[:, c2, :],
                                                inv[:, c2:c2 + 1])
                attn_h = mp.tile([H, HB], BF16, tag="ah")
                TH = T // 2
                for c2 in range(NC2):
                    aps = []
                    for th in range(2):
                        ts = slice(th * TH, (th + 1) * TH)
                        P = pbp.tile([128, H, TH], BF16, tag="P")
                        nc.vector.tensor_mul(
                            P[:], encb[:, NC2 * hf + c2, :, ts],
                            ab[:, c2, :, ts].broadcast_to((128, H, TH)))
                        ap_ = sp.tile([128, H], BF16, tag=f"pa{th}")
                        with nc.allow_low_precision(reason="bf16 attn t-reduce"):
                            nc.vector.tensor_reduce(ap_[:], P[:], axis=AX.X, op=ALU.add)
                        aps.append(ap_)
                    attnc = sp.tile([128, H], BF16, tag="attnc")
                    nc.vector.tensor_add(attnc[:], aps[0][:], aps[1][:])
                    ptr = pptr.tile([128, 128], BF16, tag="trb")
                    nc.tensor.transpose(ptr[0:H, 0:128], attnc[:], idb[:])
                    nc.scalar.copy(attn_h[:, c2 * 128:(c2 + 1) * 128], ptr[0:H, 0:128])
                attns[hf] = attn_h

            # stage gates + tail + out per half
            for hf in range(NH):
                sl = slice(hf * HB, (hf + 1) * HB)
                h_old, hb_old = hh[hf], hbb[hf]
                attn_h = attns[hf]
                prev = prevs[hf]
                ps_rz = ppg.tile([H, 2, HB], F32, tag="grz")
                for gi, g0 in ((0, 0), (1, H)):
                    nc.tensor.matmul(ps_rz[:, gi, :], WihP[:, g0:g0 + H], prev[:],
                                     start=True, stop=False)
                    nc.tensor.matmul(ps_rz[:, gi, :], WihX[:, g0:g0 + H], dxt[:, sl],
                                     start=False, stop=False)
                    nc.tensor.matmul(ps_rz[:, gi, :], WihA[:, g0:g0 + H], attn_h[:],
                                     start=False, stop=False)
                    nc.tensor.matmul(ps_rz[:, gi, :], Whh_dd[:, g0:g0 + H], hb_old[:],
                                     start=False, stop=True)
                ps_nh = ppg.tile([H, 2, HB], F32, tag="gnh")
                g0 = 2 * H
                nc.tensor.matmul(ps_nh[:, 0, :], WihP[:, g0:g0 + H], prev[:],
                                 start=True, stop=False)
                nc.tensor.matmul(ps_nh[:, 0, :], WihX[:, g0:g0 + H], dxt[:, sl],
                                 start=False, stop=False)
                nc.tensor.matmul(ps_nh[:, 0, :], WihA[:, g0:g0 + H], attn_h[:],
                                 start=False, stop=True)
                nc.tensor.matmul(ps_nh[:, 1, :], Whh_dd[:, g0:g0 + H], hb_old[:],
                                 start=True, stop=False)
                nc.tensor.matmul(ps_nh[:, 1, :], BR[0:1, H:2 * H], ones[:, sl],
                                 start=False, stop=True)
                h_new, hb_new = gru_tail(ps_rz, ps_nh, brd, bzd, bind, h_old, hf)

                # out_s = h2o @ h_new + b  -> DRAM, also feeds prev
                ps_o = pptr.tile([1, HB], F32, tag="osc")
                nc.tensor.matmul(ps_o[:], h2oWb[:], hb_new[:], start=True, stop=True)
                pv = hp.tile([1, HB], BF16, tag=f"pv{hf}")
                nc.scalar.activation(pv[:], ps_o[:], AF.Identity, bias=h2ob[:, 0:1])
                nc.sync.dma_start(out=out_d[s, sl], in_=pv[:])
                prevs[hf] = pv
                hh[hf], hbb[hf] = h_new, hb_new
    _split_excess_waits(nc)
    return nc


def _host_inputs(inputs):
    import ml_dtypes
    f = lambda a: np.ascontiguousarray(a, dtype=np.float32)
    g = lambda a: np.ascontiguousarray(np.asarray(a, dtype=np.float32),
                                       dtype=ml_dtypes.bfloat16)
    Whh_e = np.asarray(inputs["enc_Whh"]).T.copy()   # [H, 3H]
    Whh_d = np.asarray(inputs["dec_Whh"]).T.copy()
    Whh_e[:, 2 * H:3 * H] *= 0.5
    Whh_d[:, 2 * H:3 * H] *= 0.5
    shared = {
        "W1": g(inputs["s2h_W1"].T), "W2": g(inputs["s2h_W2"].T),
        "b1": f(np.asarray(inputs["s2h_b1"]).reshape(96, 1)),
        "Wih_e": g(inputs["enc_Wih"].T), "Whh_e": g(Whh_e),
        "WihP_d": g(inputs["dec_Wih"][:, 0:1].T),
        "WihX_d": g(inputs["dec_Wih"][:, 1:1 + DEC].T),
        "WihA_d": g(inputs["dec_Wih"][:, 1 + DEC:].T),
        "Whh_d": g(Whh_d),
        "UW": g(inputs["U_W"].T), "WlW": g(inputs["Wl_W"].T),
        "h2ob": f(np.asarray(inputs["h2o_b"]).reshape(1, 1)),
        "ident": f(np.eye(128)),
    }
    BI = np.zeros((H, 16), dtype=np.float32)
    ebih, ebhh = np.asarray(inputs["enc_bih"]), np.asarray(inputs["enc_bhh"])
    dbih, dbhh = np.asarray(inputs["dec_bih"]), np.asarray(inputs["dec_bhh"])
    BI[:, 0] = (ebih[0:H] + ebhh[0:H]) / 2
    BI[:, 1] = (ebih[H:2 * H] + ebhh[H:2 * H]) / 2
    BI[:, 2] = ebih[2 * H:3 * H]
    BI[:, 4] = (dbih[0:H] + dbhh[0:H]) / 2
    BI[:, 5] = (dbih[H:2 * H] + dbhh[H:2 * H]) / 2
    BI[:, 6] = dbih[2 * H:3 * H]
    BI[:, 8] = np.asarray(inputs["U_b"])
    BI[:, 9] = np.asarray(inputs["Wl_b"])
    BI[:, 10] = np.asarray(inputs["V_W"]).reshape(-1)
    BI[:, 11] = np.asarray(inputs["h2o_W"]).reshape(-1)
    BI[:, 12] = np.asarray(inputs["s2h_b2"])
    shared["BIAS"] = f(BI)
    BR = np.zeros((1, 2 * H), dtype=np.float32)
    BR[0, 0:H] = ebhh[2 * H:3 * H] / 2
    BR[0, H:2 * H] = dbhh[2 * H:3 * H] / 2
    shared["BR"] = g(BR)

    enc = np.asarray(inputs["encoder_data"])   # [T, B, ENC]
    dec = np.asarray(inputs["decoder_data"])   # [S, B, DEC]
    ann = np.asarray(inputs["ann_data"])       # [B, ANN]
    maps = []
    for i in range(NCORES):
        sl = slice(i * BS, (i + 1) * BS)
        m = dict(shared)
        m["enc"] = g(enc[:, sl, :].transpose(0, 2, 1))
        m["dec"] = g(dec[:, sl, :].transpose(0, 2, 1))
        m["ann"] = g(ann[sl, :].T)
        maps.append(m)
    return maps


def kernel(**inputs) -> np.ndarray:
    from concourse.bass_utils import run_bass_kernel_spmd
    if "nc" not in _CACHE:
        _CACHE["nc"] = _build_program()
    nc = _CACHE["nc"]
    maps = _host_inputs(inputs)
    import os
    kw = {}
    if os.environ.get("KERNEL_TRACE") == "1":
        kw = dict(trace=True, trace_cores=[0])
    res = run_bass_kernel_spmd(nc, maps, list(range(NCORES)), **kw)
    _CACHE["last_res"] = res
    outs = [np.asarray(res.results[i]["out"], dtype=np.float32) for i in range(NCORES)]
    full = np.stack(outs, axis=1)              # [S, 8, 512]
    return full.reshape(S, B, 1)


if __name__ == "__main__":
    rng = np.random.default_rng(0)
    fake = {}
    fake["ann_data"] = rng.standard_normal((B, ANN), dtype=np.float32)
    fake["encoder_data"] = rng.standard_normal((T, B, ENC), dtype=np.float32)
    fake["decoder_data"] = rng.standard_normal((S, B, DEC), dtype=np.float32)
    p = lambda *s: (rng.standard_normal(s, dtype=np.float32) * 0.08)
    fake.update({
        "s2h_W1": p(96, ANN), "s2h_b1": p(96), "s2h_W2": p(H, 96), "s2h_b2": p(H),
        "enc_Wih": p(3 * H, ENC), "enc_Whh": p(3 * H, H),
        "enc_bih": p(3 * H), "enc_bhh": p(3 * H),
        "dec_Wih": p(3 * H, 1 + DEC + H), "dec_Whh": p(3 * H, H),
        "dec_bih": p(3 * H), "dec_bhh": p(3 * H),
        "U_W": p(H, H), "U_b": p(H), "Wl_W": p(H, H), "Wl_b": p(H),
        "V_W": p(1, H), "V_b": p(1), "h2o_W": p(1, H), "h2o_b": p(1),
    })
    out = kernel(**fake)
    print("out", out.shape, out.dtype, float(np.abs(out).mean()))


# revision 16
# speedup vs baseline: 1.6839x; 1.1842x over previous
"""Trainium2 Bass kernel: GRU encoder-decoder with Bahdanau attention.

Model: B=4096, T=56 enc steps, S=28 dec steps, H=126.
Sharding: pure data parallel, batch 4096 -> 8 cores x 512.

v2 layout/engine plan:
  - All fp32 gate/proj matmuls run as float32r (single-pass PE, 1 cyc/row
    at N>=256) via AP bitcast; h state stays fp32.
  - GRU biases folded into ACT bias operands; the r*hn product uses
    pre-halved Whh_n plus a K=1 ones-row bias matmul so the hn PSUM is
    consumed directly by one scalar_tensor_tensor.
  - Decoder runs two batch halves (256 each) software-pipelined so the
    DVE/ACT serial chains of one half overlap the other's.
  - Scores: xq = tanh(Uo + Wh) as one DVE add + one ACT tanh per half;
    V-dot via M=1 matmuls (N=448, col-group packed 4x) -> PSUM -> direct
    DMA into batch-partitioned scf.
  - Softmax drops the max-subtraction (|score| <= ||V||_1 ~ 8, safe in
    fp32) and V_b (shift-invariant); 1/sum folded into alpha.
  - Attention sum: one TT mult + one bf16 tensor_reduce per half.
"""
import sys
import numpy as np

for _p in ('/opt/trn_rl_repo', '/root/.axon_site/_ro/trn_rl_repo'):
    if _p not in sys.path:
        sys.path.insert(0, _p)

from concourse import bass, tile
from concourse.vector_clock import ScopedClock

mybir = bass.mybir
F32 = mybir.dt.float32
F32R = mybir.dt.float32r
BF16 = mybir.dt.bfloat16
AF = mybir.ActivationFunctionType
ALU = mybir.AluOpType
AX = mybir.AxisListType

# ---- workaround: this walrus build allows only one embedded sync-wait on
# the Tile tail drain; spread the global-clock waits over SP nops instead.
def _patched_drain_and_barrier(self, tick_clock, wait_clock):
    nc = self.nc
    probe = nc.sync.nop()
    wait_clock.add_sem_waits(probe.ins, ScopedClock({None: tick_clock.global_clock}))
    si = probe.ins.sync_info
    waits = list(si.on_wait or []) if si is not None else []
    if si is not None:
        si.on_wait = waits[:1]
    for w in waits[1:]:
        n2 = nc.sync.nop()
        n2.ins.sync_info = mybir.SyncInfo(on_wait=[w], on_update=[])
    nc.sync.drain()
    nc.all_engine_barrier()
    popped = nc._tile_sem_poison_stack.pop()
    assert popped is self._sem_poison
    nc.clear_and_free_semaphores(list(self.sems.allocated().values()))
    nc.all_engine_barrier()

tile.TileContext._drain_and_barrier = _patched_drain_and_barrier


def _split_excess_waits(nc):
    """This walrus build allows 1 embedded sync-wait per instruction; move
    extras onto same-engine nops inserted just before the instruction."""
    cnt = 0
    for _, bassbb in list(nc.bb_map.items()):
        bb = bassbb.bb if hasattr(bassbb, "bb") else bassbb
        il = bb.instructions
        i = 0
        while i < len(il):
            inst = il[i]
            si = inst.sync_info
            if si is not None and si.on_wait and len(si.on_wait) > 1:
                extra = list(si.on_wait[:-1])
                si.on_wait = [si.on_wait[-1]]
                for w in extra:
                    cnt += 1
                    nop = mybir.InstNoOp(name=f"wfix-{cnt}", ins=[], outs=[])
                    nop.engine = inst.engine
                    nop.sync_info = mybir.SyncInfo(on_wait=[w], on_update=[])
                    il.insert(i, nop)
                    i += 1
            i += 1
    return cnt

B, T, S = 4096, 56, 28
H, ANN, ENC, DEC = 126, 30, 20, 15
NCORES = 8
BS = B // NCORES          # 512 batch per core
NH = 2                    # decoder batch halves (software pipeline)
HB = BS // NH             # 256 batch per half
NC2 = HB // 128           # 2 chunks of 128 per half
TQ = 14                   # t-block for the V-dot matmuls
BBLK = 32                 # batch block per V-dot matmul

_CACHE = {}


def _build_program():
    import os
    kt = int(os.environ.get("K_T", T))
    ks = int(os.environ.get("K_S", S))
    nc = bass.Bass()

    di = lambda name, shape, dt=F32: nc.declare_dram_parameter(name, list(shape), dt, isOutput=False)
    enc_d = di("enc", (T, ENC, BS), BF16)
    dec_d = di("dec", (S, DEC, BS), BF16)
    ann_d = di("ann", (ANN, BS), BF16)
    W1_d = di("W1", (ANN, 96), BF16)
    W2_d = di("W2", (96, H), BF16)
    b1_d = di("b1", (96, 1))
    Wih_e_d = di("Wih_e", (ENC, 3 * H), BF16)
    Whh_e_d = di("Whh_e", (H, 3 * H), BF16)
    WihP_d_d = di("WihP_d", (1, 3 * H), BF16)
    WihX_d_d = di("WihX_d", (DEC, 3 * H), BF16)
    WihA_d_d = di("WihA_d", (H, 3 * H), BF16)
    Whh_d_d = di("Whh_d", (H, 3 * H), BF16)
    UW_d = di("UW", (H, H), BF16)
    WlW_d = di("WlW", (H, H), BF16)
    B_d = di("BIAS", (H, 16))
    BR_d = di("BR", (1, 2 * H), BF16)
    h2ob_d = di("h2ob", (1, 1))
    id_d = di("ident", (128, 128))
    out_d = nc.declare_dram_parameter("out", [S, BS], BF16, isOutput=True)

    from contextlib import ExitStack
    with tile.TileContext(nc) as tc, ExitStack() as es:
        cp = es.enter_context(tc.tile_pool(name="const", bufs=1))
        sp = es.enter_context(tc.tile_pool(name="sb", bufs=2))
        hp = es.enter_context(tc.tile_pool(name="hs", bufs=4))
        xqp = es.enter_context(tc.tile_pool(name="xq", bufs=2))
        pbp = es.enter_context(tc.tile_pool(name="pb", bufs=2))
        ppg = es.enter_context(tc.tile_pool(name="psg", bufs=2, space="PSUM"))
        ppw = es.enter_context(tc.tile_pool(name="psw", bufs=1, space="PSUM"))
        ppsc = es.enter_context(tc.tile_pool(name="pssc", bufs=1, space="PSUM"))
        pptr = es.enter_context(tc.tile_pool(name="pstr", bufs=1, space="PSUM"))
        gp = es.enter_context(tc.tile_pool(name="gates", bufs=8))
        mp = es.enter_context(tc.tile_pool(name="misc", bufs=2))

        def cload(dram, shape, dtype=F32):
            t_ = cp.tile(list(shape), dtype, tag="c_" + dram.name)
            nc.sync.dma_start(out=t_[:], in_=dram[:])
            return t_

        W1 = cload(W1_d, (ANN, 96), BF16)
        W2 = cload(W2_d, (96, H), BF16)
        b1 = cload(b1_d, (96, 1))
        Wih_e = cload(Wih_e_d, (ENC, 3 * H), BF16)
        Whh_e = cload(Whh_e_d, (H, 3 * H), BF16)
        WihP = cload(WihP_d_d, (1, 3 * H), BF16)
        WihX = cload(WihX_d_d, (DEC, 3 * H), BF16)
        WihA = cload(WihA_d_d, (H, 3 * H), BF16)
        Whh_dd = cload(Whh_d_d, (H, 3 * H), BF16)
        UW = cload(UW_d, (H, H), BF16)
        WlW = cload(WlW_d, (H, H), BF16)
        BI = cload(B_d, (H, 16))
        BR = cload(BR_d, (1, 2 * H), BF16)
        h2ob = cload(h2ob_d, (1, 1))
        idf = cload(id_d, (128, 128))
        idb = cp.tile([128, 128], BF16, tag="idb")
        nc.vector.tensor_copy(idb[:], idf[:])
        Vb = cp.tile([H, 1], BF16, tag="Vb")
        nc.vector.tensor_copy(Vb[:], BI[:, 10:11])
        h2oWb = cp.tile([H, 1], BF16, tag="h2oWb")
        nc.vector.tensor_copy(h2oWb[:], BI[:, 11:12])
        ones = cp.tile([1, BS], BF16, tag="ones")
        nc.vector.memset(ones[:], 1.0)

        # persistent big tensors
        Uo = cp.tile([H, T, BS], BF16, tag="Uo")            # 57.3 KB/part
        encb = cp.tile([128, NH * NC2, H, T], BF16, tag="encb")  # 56.4 KB/part

        # bias columns (r/z biases pre-halved on host)
        bre, bze, bine = BI[:, 0:1], BI[:, 1:2], BI[:, 2:3]
        brd, bzd, bind = BI[:, 4:5], BI[:, 5:6], BI[:, 6:7]
        Ub, Wlb, h2oW, b2 = BI[:, 8:9], BI[:, 9:10], BI[:, 11:12], BI[:, 12:13]

        # ---------------- phase A: static -> h0 ----------------
        annt = sp.tile([ANN, BS], BF16, tag="x")
        nc.sync.dma_start(out=annt[:], in_=ann_d[:])
        ps96 = ppg.tile([96, BS], F32, tag="grz")
        nc.tensor.matmul(ps96[:], W1[:], annt[:], start=True, stop=True)
        hid1 = sp.tile([96, BS], BF16, tag="hid")
        nc.scalar.activation(hid1[:], ps96[:], AF.Relu, bias=b1[:, 0:1])
        psh = ppg.tile([H, BS], F32, tag="grz")
        nc.tensor.matmul(psh[:], W2[:], hid1[:], start=True, stop=True)
        hh = [hp.tile([H, HB], F32, tag=f"h{hf}", name=f"h{hf}") for hf in range(NH)]
        hbb = [hp.tile([H, HB], BF16, tag=f"hb{hf}", name=f"hb{hf}") for hf in range(NH)]
        for hf in range(NH):
            nc.scalar.activation(hh[hf][:], psh[:, hf * HB:(hf + 1) * HB],
                                 AF.Identity, bias=b2)
            nc.scalar.activation(hbb[hf][:], psh[:, hf * HB:(hf + 1) * HB],
                                 AF.Identity, bias=b2)

        # one GRU tail, consumes gate psums -> h_new (per half)
        # ps_rz: [H,2,HB] (r,z); ps_nh: [H,2,HB] (ni, hn_half incl bias)
        def gru_tail(ps_rz, ps_nh, br_, bz_, bin_, h_old, hf):
            th_r = gp.tile([H, HB], F32, tag="gate")
            nc.scalar.activation(th_r[:], ps_rz[:, 0, :], AF.Tanh, bias=br_, scale=0.5)
            th_z = gp.tile([H, HB], F32, tag="gate")
            nc.scalar.activation(th_z[:], ps_rz[:, 1, :], AF.Tanh, bias=bz_, scale=0.5)
            tmp = gp.tile([H, HB], F32, tag="gate")
            nc.vector.scalar_tensor_tensor(tmp[:], th_r[:], 1.0, ps_nh[:, 1, :],
                                           ALU.add, ALU.mult)
            pre = gp.tile([H, HB], F32, tag="gate")
            nc.vector.tensor_add(pre[:], tmp[:], ps_nh[:, 0, :])
            n_ = gp.tile([H, HB], F32, tag="gate")
            nc.scalar.activation(n_[:], pre[:], AF.Tanh, bias=bin_)
            d_ = gp.tile([H, HB], F32, tag="gate")
            nc.vector.tensor_sub(d_[:], n_[:], h_old[:])
            v1 = gp.tile([H, HB], F32, tag="gate")
            nc.vector.scalar_tensor_tensor(v1[:], th_z[:], -1.0, d_[:], ALU.add, ALU.mult)
            h_new = hp.tile([H, HB], F32, tag=f"h{hf}")
            nc.vector.scalar_tensor_tensor(h_new[:], v1[:], -0.5, h_old[:],
                                           ALU.mult, ALU.add)
            hb_new = hp.tile([H, HB], BF16, tag=f"hb{hf}", name=f"hbn{hf}")
            nc.vector.tensor_copy(hb_new[:], h_new[:])
            return h_new, hb_new

        # ---------------- phase B: encoder ----------------
        for t in range(kt):
            xt = sp.tile([ENC, BS], BF16, tag="x")
            nc.sync.dma_start(out=xt[:], in_=enc_d[t])
            for hf in range(NH):
                sl = slice(hf * HB, (hf + 1) * HB)
                h_old, hb_old = hh[hf], hbb[hf]
                ps_rz = ppg.tile([H, 2, HB], F32, tag="grz")
                nc.tensor.matmul(ps_rz[:, 0, :], Wih_e[:, 0:H], xt[:, sl],
                                 start=True, stop=False)
                nc.tensor.matmul(ps_rz[:, 0, :], Whh_e[:, 0:H], hb_old[:],
                                 start=False, stop=True)
                nc.tensor.matmul(ps_rz[:, 1, :], Wih_e[:, H:2 * H], xt[:, sl],
                                 start=True, stop=False)
                nc.tensor.matmul(ps_rz[:, 1, :], Whh_e[:, H:2 * H], hb_old[:],
                                 start=False, stop=True)
                ps_nh = ppg.tile([H, 2, HB], F32, tag="gnh")
                nc.tensor.matmul(ps_nh[:, 0, :], Wih_e[:, 2 * H:3 * H], xt[:, sl],
                                 start=True, stop=True)
                nc.tensor.matmul(ps_nh[:, 1, :], Whh_e[:, 2 * H:3 * H], hb_old[:],
                                 start=True, stop=False)
                nc.tensor.matmul(ps_nh[:, 1, :], BR[0:1, 0:H], ones[:, sl],
                                 start=False, stop=True)
                h_new, hb_new = gru_tail(ps_rz, ps_nh, bre, bze, bine, h_old, hf)

                # Uo[:, t, sl] = U @ h_new + Ub   (bf16)
                ps_uo = ppw.tile([H, HB], F32, tag="wh")
                nc.tensor.matmul(ps_uo[:], UW[:], hb_new[:], start=True, stop=True)
                nc.scalar.activation(Uo[:, t, sl], ps_uo[:], AF.Identity, bias=Ub)

                # encb[:, 2hf+c2, :, t] = h_new.T chunks (bf16)
                for c2 in range(NC2):
                    ptr = pptr.tile([128, 128], BF16, tag="trb")
                    nc.tensor.transpose(ptr[0:128, 0:H],
                                        hb_new[:, c2 * 128:(c2 + 1) * 128],
                                        idb[0:H, 0:H])
                    nc.scalar.copy(encb[:, NC2 * hf + c2, :, t], ptr[0:128, 0:H])
                hh[hf], hbb[hf] = h_new, hb_new

        # ---------------- phase C: decoder ----------------
        prevs = []
        for hf in range(NH):
            pv = hp.tile([1, HB], BF16, tag=f"pv{hf}")
            nc.sync.dma_start(out=pv[:], in_=enc_d[T - 1, 0:1, hf * HB:(hf + 1) * HB])
            prevs.append(pv)

        for s in range(ks):
            dxt = sp.tile([DEC, BS], BF16, tag="dx")
            nc.sync.dma_start(out=dxt[:], in_=dec_d[s])

            scfs, attns = {}, {}
            # stage W + scores pipeline (t-chunks of TQ) per half
            for hf in range(NH):
                sl = slice(hf * HB, (hf + 1) * HB)
                ps_wh = ppw.tile([H, HB], F32, tag="wh")
                nc.tensor.matmul(ps_wh[:], WlW[:], hbb[hf][:], start=True, stop=True)
                wh = mp.tile([H, 1, HB], BF16, tag="whb")
                nc.scalar.activation(wh[:, 0, :], ps_wh[:], AF.Identity, bias=Wlb)
                scf = sp.tile([128, NC2, T], F32, tag="sco")
                for q in range(T // TQ):
                    tq = slice(q * TQ, (q + 1) * TQ)
                    xq = xqp.tile([H, TQ, HB], BF16, tag="xt")
                    nc.vector.tensor_add(xq[:], Uo[:, tq, sl],
                                         wh[:].broadcast_to((H, TQ, HB)))
                    nc.scalar.activation(xq[:], xq[:], AF.Tanh)
                    # rhs iterates t-outer, b-inner so the moving stream reads
                    # contiguous 32-col bursts from xq's [t, b] free layout
                    for c2 in range(NC2):
                        pssc = ppsc.tile([128, TQ, BBLK], F32, tag="sc")
                        for j in range(4):
                            b0 = c2 * 128 + j * BBLK
                            nc.tensor.matmul(pssc[BBLK * j:BBLK * j + 1, :, :], Vb[:],
                                             xq[:, :, b0:b0 + BBLK],
                                             start=True, stop=True,
                                             tile_position=(0, BBLK * j))
                        sstg = mp.tile([128, BBLK, TQ], F32, tag="sst")
                        nc.scalar.copy(sstg[:], pssc[:].transpose([0, 2, 1]))
                        nc.sync.dma_start(out=scf[:, c2, tq],
                                          in_=sstg[0:128:BBLK])
                scfs[hf] = scf

            # stage softmax (no max-subtraction; scores bounded by ||V||_1)
            # + attention weighted sum + transpose back
            for hf in range(NH):
                scf = scfs[hf]
                expo = sp.tile([128, NC2, T], F32, tag="expo")
                nc.scalar.activation(expo[:], scf[:], AF.Exp)
                sm = sp.tile([128, NC2], F32, tag="red")
                nc.vector.tensor_reduce(sm[:], expo[:], axis=AX.X, op=ALU.add)
                inv = sp.tile([128, NC2], F32, tag="red2")
                nc.vector.reciprocal(inv[:], sm[:])
                ab = sp.tile([128, NC2, 1, T], BF16, tag="ab")
                for c2 in range(NC2):
                    nc.vector.tensor_scalar_mul(ab[:, c2, 0, :], expo[:, c2, :],
                                                inv[:, c2:c2 + 1])
                attn_h = mp.tile([H, HB], BF16, tag="ah")
                TH = T // 2
                for c2 in range(NC2):
                    # alpha-weighted products, then in-place bf16 fold tree
                    # (TT adds run 2x; TensorReduce only 1x) down to 7 t-slots
                    P1 = pbp.tile([128, H, TH], BF16, tag="P")
                    nc.vector.tensor_mul(
                        P1[:], encb[:, NC2 * hf + c2, :, 0:TH],
                        ab[:, c2, :, 0:TH].broadcast_to((128, H, TH)))
                    P2 = pbp.tile([128, H, TH], BF16, tag="P")
                    nc.vector.tensor_mul(
                        P2[:], encb[:, NC2 * hf + c2, :, TH:T],
                        ab[:, c2, :, TH:T].broadcast_to((128, H, TH)))
                    nc.vector.tensor_add(P1[:], P1[:], P2[:])
                    nc.vector.tensor_add(P1[:, :, 0:14], P1[:, :, 0:14], P1[:, :, 14:28])
                    nc.vector.tensor_add(P1[:, :, 0:7], P1[:, :, 0:7], P1[:, :, 7:14])
                    attnc = sp.tile([128, H], BF16, tag="attnc")
                    with nc.allow_low_precision(reason="bf16 attn t-reduce"):
                        nc.vector.tensor_reduce(attnc[:], P1[:, :, 0:7], axis=AX.X,
                                                op=ALU.add)
                    ptr = pptr.tile([128, 128], BF16, tag="trb")
                    nc.tensor.transpose(ptr[0:H, 0:128], attnc[:], idb[:])
                    nc.scalar.copy(attn_h[:, c2 * 128:(c2 + 1) * 128], ptr[0:H, 0:128])
                attns[hf] = attn_h

            # stage gates + tail + out per half
            for hf in range(NH):
                sl = slice(hf * HB, (hf + 1) * HB)
                h_old, hb_old = hh[hf], hbb[hf]
                attn_h = attns[hf]
                prev = prevs[hf]
                ps_rz = ppg.tile([H, 2, HB], F32, tag="grz")
                for gi, g0 in ((0, 0), (1, H)):
                    nc.tensor.matmul(ps_rz[:, gi, :], WihP[:, g0:g0 + H], prev[:],
                                     start=True, stop=False)
                    nc.tensor.matmul(ps_rz[:, gi, :], WihX[:, g0:g0 + H], dxt[:, sl],
                                     start=False, stop=False)
                    nc.tensor.matmul(ps_rz[:, gi, :], WihA[:, g0:g0 + H], attn_h[:],
                                     start=False, stop=False)
                    nc.tensor.matmul(ps_rz[:, gi, :], Whh_dd[:, g0:g0 + H], hb_old[:],
                                     start=False, stop=True)
                ps_nh = ppg.tile([H, 2, HB], F32, tag="gnh")
                g0 = 2 * H
                nc.tensor.matmul(ps_nh[:, 0, :], WihP[:, g0:g0 + H], prev[:],
                                 start=True, stop=False)
                nc.tensor.matmul(ps_nh[:, 0, :], WihX[:, g0:g0 + H], dxt[:, sl],
                                 start=False, stop=False)
                nc.tensor.matmul(ps_nh[:, 0, :], WihA[:, g0:g0 + H], attn_h[:],
                                 start=False, stop=True)
                nc.tensor.matmul(ps_nh[:, 1, :], Whh_dd[:, g0:g0 + H], hb_old[:],
                                 start=True, stop=False)
                nc.tensor.matmul(ps_nh[:, 1, :], BR[0:1, H:2 * H], ones[:, sl],
                                 start=False, stop=True)
                h_new, hb_new = gru_tail(ps_rz, ps_nh, brd, bzd, bind, h_old, hf)

                # out_s = h2o @ h_new + b  -> DRAM, also feeds prev
                ps_o = pptr.tile([1, HB], F32, tag="osc")
                nc.tensor.matmul(ps_o[:], h2oWb[:], hb_new[:], start=True, stop=True)
                pv = hp.tile([1, HB], BF16, tag=f"pv{hf}")
                nc.scalar.activation(pv[:], ps_o[:], AF.Identity, bias=h2ob[:, 0:1])
                nc.sync.dma_start(out=out_d[s, sl], in_=pv[:])
                prevs[hf] = pv
                hh[hf], hbb[hf] = h_new, hb_new
    _split_excess_waits(nc)
    return nc


def _host_inputs(inputs):
    import ml_dtypes
    f = lambda a: np.ascontiguousarray(a, dtype=np.float32)
    g = lambda a: np.ascontiguousarray(np.asarray(a, dtype=np.float32),
                                       dtype=ml_dtypes.bfloat16)
    Whh_e = np.asarray(inputs["enc_Whh"]).T.copy()   # [H, 3H]
    Whh_d = np.asarray(inputs["dec_Whh"]).T.copy()
    Whh_e[:, 2 * H:3 * H] *= 0.5
    Whh_d[:, 2 * H:3 * H] *= 0.5
    shared = {
        "W1": g(inputs["s2h_W1"].T), "W2": g(inputs["s2h_W2"].T),
        "b1": f(np.asarray(inputs["s2h_b1"]).reshape(96, 1)),
        "Wih_e": g(inputs["enc_Wih"].T), "Whh_e": g(Whh_e),
        "WihP_d": g(inputs["dec_Wih"][:, 0:1].T),
        "WihX_d": g(inputs["dec_Wih"][:, 1:1 + DEC].T),
        "WihA_d": g(inputs["dec_Wih"][:, 1 + DEC:].T),
        "Whh_d": g(Whh_d),
        "UW": g(inputs["U_W"].T), "WlW": g(inputs["Wl_W"].T),
        "h2ob": f(np.asarray(inputs["h2o_b"]).reshape(1, 1)),
        "ident": f(np.eye(128)),
    }
    BI = np.zeros((H, 16), dtype=np.float32)
    ebih, ebhh = np.asarray(inputs["enc_bih"]), np.asarray(inputs["enc_bhh"])
    dbih, dbhh = np.asarray(inputs["dec_bih"]), np.asarray(inputs["dec_bhh"])
    BI[:, 0] = (ebih[0:H] + ebhh[0:H]) / 2
    BI[:, 1] = (ebih[H:2 * H] + ebhh[H:2 * H]) / 2
    BI[:, 2] = ebih[2 * H:3 * H]
    BI[:, 4] = (dbih[0:H] + dbhh[0:H]) / 2
    BI[:, 5] = (dbih[H:2 * H] + dbhh[H:2 * H]) / 2
    BI[:, 6] = dbih[2 * H:3 * H]
    BI[:, 8] = np.asarray(inputs["U_b"])
    BI[:, 9] = np.asarray(inputs["Wl_b"])
    BI[:, 10] = np.asarray(inputs["V_W"]).reshape(-1)
    BI[:, 11] = np.asarray(inputs["h2o_W"]).reshape(-1)
    BI[:, 12] = np.asarray(inputs["s2h_b2"])
    shared["BIAS"] = f(BI)
    BR = np.zeros((1, 2 * H), dtype=np.float32)
    BR[0, 0:H] = ebhh[2 * H:3 * H] / 2
    BR[0, H:2 * H] = dbhh[2 * H:3 * H] / 2
    shared["BR"] = g(BR)

    enc = np.asarray(inputs["encoder_data"])   # [T, B, ENC]
    dec = np.asarray(inputs["decoder_data"])   # [S, B, DEC]
    ann = np.asarray(inputs["ann_data"])       # [B, ANN]
    maps = []
    for i in range(NCORES):
        sl = slice(i * BS, (i + 1) * BS)
        m = dict(shared)
        m["enc"] = g(enc[:, sl, :].transpose(0, 2, 1))
        m["dec"] = g(dec[:, sl, :].transpose(0, 2, 1))
        m["ann"] = g(ann[sl, :].T)
        maps.append(m)
    return maps


def kernel(**inputs) -> np.ndarray:
    from concourse.bass_utils import run_bass_kernel_spmd
    if "nc" not in _CACHE:
        _CACHE["nc"] = _build_program()
    nc = _CACHE["nc"]
    maps = _host_inputs(inputs)
    import os
    kw = {}
    if os.environ.get("KERNEL_TRACE") == "1":
        kw = dict(trace=True, trace_cores=[0])
    res = run_bass_kernel_spmd(nc, maps, list(range(NCORES)), **kw)
    _CACHE["last_res"] = res
    outs = [np.asarray(res.results[i]["out"], dtype=np.float32) for i in range(NCORES)]
    full = np.stack(outs, axis=1)              # [S, 8, 512]
    return full.reshape(S, B, 1)


if __name__ == "__main__":
    rng = np.random.default_rng(0)
    fake = {}
    fake["ann_data"] = rng.standard_normal((B, ANN), dtype=np.float32)
    fake["encoder_data"] = rng.standard_normal((T, B, ENC), dtype=np.float32)
    fake["decoder_data"] = rng.standard_normal((S, B, DEC), dtype=np.float32)
    p = lambda *s: (rng.standard_normal(s, dtype=np.float32) * 0.08)
    fake.update({
        "s2h_W1": p(96, ANN), "s2h_b1": p(96), "s2h_W2": p(H, 96), "s2h_b2": p(H),
        "enc_Wih": p(3 * H, ENC), "enc_Whh": p(3 * H, H),
        "enc_bih": p(3 * H), "enc_bhh": p(3 * H),
        "dec_Wih": p(3 * H, 1 + DEC + H), "dec_Whh": p(3 * H, H),
        "dec_bih": p(3 * H), "dec_bhh": p(3 * H),
        "U_W": p(H, H), "U_b": p(H), "Wl_W": p(H, H), "Wl_b": p(H),
        "V_W": p(1, H), "V_b": p(1), "h2o_W": p(1, H), "h2o_b": p(1),
    })
    out = kernel(**fake)
    print("out", out.shape, out.dtype, float(np.abs(out).mean()))


# revision 17
# speedup vs baseline: 1.6939x; 1.0060x over previous
"""Trainium2 Bass kernel: GRU encoder-decoder with Bahdanau attention.

Model: B=4096, T=56 enc steps, S=28 dec steps, H=126.
Sharding: pure data parallel, batch 4096 -> 8 cores x 512.

v2 layout/engine plan:
  - All fp32 gate/proj matmuls run as float32r (single-pass PE, 1 cyc/row
    at N>=256) via AP bitcast; h state stays fp32.
  - GRU biases folded into ACT bias operands; the r*hn product uses
    pre-halved Whh_n plus a K=1 ones-row bias matmul so the hn PSUM is
    consumed directly by one scalar_tensor_tensor.
  - Decoder runs two batch halves (256 each) software-pipelined so the
    DVE/ACT serial chains of one half overlap the other's.
  - Scores: xq = tanh(Uo + Wh) as one DVE add + one ACT tanh per half;
    V-dot via M=1 matmuls (N=448, col-group packed 4x) -> PSUM -> direct
    DMA into batch-partitioned scf.
  - Softmax drops the max-subtraction (|score| <= ||V||_1 ~ 8, safe in
    fp32) and V_b (shift-invariant); 1/sum folded into alpha.
  - Attention sum: one TT mult + one bf16 tensor_reduce per half.
"""
import sys
import numpy as np

for _p in ('/opt/trn_rl_repo', '/root/.axon_site/_ro/trn_rl_repo'):
    if _p not in sys.path:
        sys.path.insert(0, _p)

from concourse import bass, tile
from concourse.vector_clock import ScopedClock

mybir = bass.mybir
F32 = mybir.dt.float32
F32R = mybir.dt.float32r
BF16 = mybir.dt.bfloat16
AF = mybir.ActivationFunctionType
ALU = mybir.AluOpType
AX = mybir.AxisListType

# ---- workaround: this walrus build allows only one embedded sync-wait on
# the Tile tail drain; spread the global-clock waits over SP nops instead.
def _patched_drain_and_barrier(self, tick_clock, wait_clock):
    nc = self.nc
    probe = nc.sync.nop()
    wait_clock.add_sem_waits(probe.ins, ScopedClock({None: tick_clock.global_clock}))
    si = probe.ins.sync_info
    waits = list(si.on_wait or []) if si is not None else []
    if si is not None:
        si.on_wait = waits[:1]
    for w in waits[1:]:
        n2 = nc.sync.nop()
        n2.ins.sync_info = mybir.SyncInfo(on_wait=[w], on_update=[])
    nc.sync.drain()
    nc.all_engine_barrier()
    popped = nc._tile_sem_poison_stack.pop()
    assert popped is self._sem_poison
    nc.clear_and_free_semaphores(list(self.sems.allocated().values()))
    nc.all_engine_barrier()

tile.TileContext._drain_and_barrier = _patched_drain_and_barrier


def _split_excess_waits(nc):
    """This walrus build allows 1 embedded sync-wait per instruction; move
    extras onto same-engine nops inserted just before the instruction."""
    cnt = 0
    for _, bassbb in list(nc.bb_map.items()):
        bb = bassbb.bb if hasattr(bassbb, "bb") else bassbb
        il = bb.instructions
        i = 0
        while i < len(il):
            inst = il[i]
            si = inst.sync_info
            if si is not None and si.on_wait and len(si.on_wait) > 1:
                extra = list(si.on_wait[:-1])
                si.on_wait = [si.on_wait[-1]]
                for w in extra:
                    cnt += 1
                    nop = mybir.InstNoOp(name=f"wfix-{cnt}", ins=[], outs=[])
                    nop.engine = inst.engine
                    nop.sync_info = mybir.SyncInfo(on_wait=[w], on_update=[])
                    il.insert(i, nop)
                    i += 1
            i += 1
    return cnt

B, T, S = 4096, 56, 28
H, ANN, ENC, DEC = 126, 30, 20, 15
NCORES = 8
BS = B // NCORES          # 512 batch per core
NH = 2                    # decoder batch halves (software pipeline)
HB = BS // NH             # 256 batch per half
NC2 = HB // 128           # 2 chunks of 128 per half
TQ = 14                   # t-block for the V-dot matmuls
BBLK = 32                 # batch block per V-dot matmul

_CACHE = {}


def _build_program():
    import os
    kt = int(os.environ.get("K_T", T))
    ks = int(os.environ.get("K_S", S))
    nc = bass.Bass()

    di = lambda name, shape, dt=F32: nc.declare_dram_parameter(name, list(shape), dt, isOutput=False)
    enc_d = di("enc", (T, ENC, BS), BF16)
    dec_d = di("dec", (S, DEC, BS), BF16)
    ann_d = di("ann", (ANN, BS), BF16)
    W1_d = di("W1", (ANN, 96), BF16)
    W2_d = di("W2", (96, H), BF16)
    b1_d = di("b1", (96, 1))
    Wih_e_d = di("Wih_e", (ENC, 3 * H), BF16)
    Whh_e_d = di("Whh_e", (H, 3 * H), BF16)
    WihP_d_d = di("WihP_d", (1, 3 * H), BF16)
    WihX_d_d = di("WihX_d", (DEC, 3 * H), BF16)
    WihA_d_d = di("WihA_d", (H, 3 * H), BF16)
    Whh_d_d = di("Whh_d", (H, 3 * H), BF16)
    UW_d = di("UW", (H, H), BF16)
    WlW_d = di("WlW", (H, H), BF16)
    B_d = di("BIAS", (H, 16))
    BR_d = di("BR", (1, 2 * H), BF16)
    h2ob_d = di("h2ob", (1, 1))
    id_d = di("ident", (128, 128))
    out_d = nc.declare_dram_parameter("out", [S, BS], BF16, isOutput=True)

    from contextlib import ExitStack
    with tile.TileContext(nc) as tc, ExitStack() as es:
        cp = es.enter_context(tc.tile_pool(name="const", bufs=1))
        sp = es.enter_context(tc.tile_pool(name="sb", bufs=2))
        hp = es.enter_context(tc.tile_pool(name="hs", bufs=4))
        xqp = es.enter_context(tc.tile_pool(name="xq", bufs=2))
        pbp = es.enter_context(tc.tile_pool(name="pb", bufs=2))
        ppg = es.enter_context(tc.tile_pool(name="psg", bufs=2, space="PSUM"))
        ppw = es.enter_context(tc.tile_pool(name="psw", bufs=1, space="PSUM"))
        ppsc = es.enter_context(tc.tile_pool(name="pssc", bufs=1, space="PSUM"))
        pptr = es.enter_context(tc.tile_pool(name="pstr", bufs=1, space="PSUM"))
        gp = es.enter_context(tc.tile_pool(name="gates", bufs=8))
        mp = es.enter_context(tc.tile_pool(name="misc", bufs=2))

        def cload(dram, shape, dtype=F32):
            t_ = cp.tile(list(shape), dtype, tag="c_" + dram.name)
            nc.sync.dma_start(out=t_[:], in_=dram[:])
            return t_

        W1 = cload(W1_d, (ANN, 96), BF16)
        W2 = cload(W2_d, (96, H), BF16)
        b1 = cload(b1_d, (96, 1))
        Wih_e = cload(Wih_e_d, (ENC, 3 * H), BF16)
        Whh_e = cload(Whh_e_d, (H, 3 * H), BF16)
        WihP = cload(WihP_d_d, (1, 3 * H), BF16)
        WihX = cload(WihX_d_d, (DEC, 3 * H), BF16)
        WihA = cload(WihA_d_d, (H, 3 * H), BF16)
        Whh_dd = cload(Whh_d_d, (H, 3 * H), BF16)
        UW = cload(UW_d, (H, H), BF16)
        WlW = cload(WlW_d, (H, H), BF16)
        BI = cload(B_d, (H, 16))
        BR = cload(BR_d, (1, 2 * H), BF16)
        h2ob = cload(h2ob_d, (1, 1))
        idf = cload(id_d, (128, 128))
        idb = cp.tile([128, 128], BF16, tag="idb")
        nc.vector.tensor_copy(idb[:], idf[:])
        Vb = cp.tile([H, 1], BF16, tag="Vb")
        nc.vector.tensor_copy(Vb[:], BI[:, 10:11])
        h2oWb = cp.tile([H, 1], BF16, tag="h2oWb")
        nc.vector.tensor_copy(h2oWb[:], BI[:, 11:12])
        ones = cp.tile([1, BS], BF16, tag="ones")
        nc.vector.memset(ones[:], 1.0)

        # persistent big tensors
        Uo = cp.tile([H, T, BS], BF16, tag="Uo")            # 57.3 KB/part
        encb = cp.tile([128, NH * NC2, H, T], BF16, tag="encb")  # 56.4 KB/part

        # bias columns (r/z biases pre-halved on host)
        bre, bze, bine = BI[:, 0:1], BI[:, 1:2], BI[:, 2:3]
        brd, bzd, bind = BI[:, 4:5], BI[:, 5:6], BI[:, 6:7]
        Ub, Wlb, h2oW, b2 = BI[:, 8:9], BI[:, 9:10], BI[:, 11:12], BI[:, 12:13]

        # ---------------- phase A: static -> h0 ----------------
        annt = sp.tile([ANN, BS], BF16, tag="x")
        nc.sync.dma_start(out=annt[:], in_=ann_d[:])
        ps96 = ppg.tile([96, BS], F32, tag="grz")
        nc.tensor.matmul(ps96[:], W1[:], annt[:], start=True, stop=True)
        hid1 = sp.tile([96, BS], BF16, tag="hid")
        nc.scalar.activation(hid1[:], ps96[:], AF.Relu, bias=b1[:, 0:1])
        psh = ppg.tile([H, BS], F32, tag="grz")
        nc.tensor.matmul(psh[:], W2[:], hid1[:], start=True, stop=True)
        hbb = [hp.tile([H, HB], BF16, tag=f"hb{hf}", name=f"hb{hf}") for hf in range(NH)]
        for hf in range(NH):
            nc.scalar.activation(hbb[hf][:], psh[:, hf * HB:(hf + 1) * HB],
                                 AF.Identity, bias=b2)

        # one GRU tail, consumes gate psums -> h_new (per half)
        # ps_rz: [H,2,HB] (r,z); ps_nh: [H,2,HB] (ni, hn_half incl bias)
        def gru_tail(ps_rz, ps_nh, br_, bz_, bin_, hb_old, hf):
            th_r = gp.tile([H, HB], F32, tag="gate")
            nc.scalar.activation(th_r[:], ps_rz[:, 0, :], AF.Tanh, bias=br_, scale=0.5)
            th_z = gp.tile([H, HB], BF16, tag="gatez")
            nc.scalar.activation(th_z[:], ps_rz[:, 1, :], AF.Tanh, bias=bz_, scale=0.5)
            tmp = gp.tile([H, HB], F32, tag="gate")
            nc.vector.scalar_tensor_tensor(tmp[:], th_r[:], 1.0, ps_nh[:, 1, :],
                                           ALU.add, ALU.mult)
            pre = gp.tile([H, HB], F32, tag="gate")
            nc.vector.tensor_add(pre[:], tmp[:], ps_nh[:, 0, :])
            n_ = gp.tile([H, HB], BF16, tag="gatez")
            nc.scalar.activation(n_[:], pre[:], AF.Tanh, bias=bin_)
            d_ = gp.tile([H, HB], BF16, tag="gatez")
            nc.vector.tensor_sub(d_[:], n_[:], hb_old[:])
            v1 = gp.tile([H, HB], BF16, tag="gatez")
            nc.vector.scalar_tensor_tensor(v1[:], th_z[:], -1.0, d_[:], ALU.add, ALU.mult)
            hb_new = hp.tile([H, HB], BF16, tag=f"hb{hf}", name=f"hbn{hf}")
            nc.vector.scalar_tensor_tensor(hb_new[:], v1[:], -0.5, hb_old[:],
                                           ALU.mult, ALU.add)
            return hb_new

        # ---------------- phase B: encoder ----------------
        for t in range(kt):
            xt = sp.tile([ENC, BS], BF16, tag="x")
            nc.sync.dma_start(out=xt[:], in_=enc_d[t])
            for hf in range(NH):
                sl = slice(hf * HB, (hf + 1) * HB)
                hb_old = hbb[hf]
                ps_rz = ppg.tile([H, 2, HB], F32, tag="grz")
                nc.tensor.matmul(ps_rz[:, 0, :], Wih_e[:, 0:H], xt[:, sl],
                                 start=True, stop=False)
                nc.tensor.matmul(ps_rz[:, 0, :], Whh_e[:, 0:H], hb_old[:],
                                 start=False, stop=True)
                nc.tensor.matmul(ps_rz[:, 1, :], Wih_e[:, H:2 * H], xt[:, sl],
                                 start=True, stop=False)
                nc.tensor.matmul(ps_rz[:, 1, :], Whh_e[:, H:2 * H], hb_old[:],
                                 start=False, stop=True)
                ps_nh = ppg.tile([H, 2, HB], F32, tag="gnh")
                nc.tensor.matmul(ps_nh[:, 0, :], Wih_e[:, 2 * H:3 * H], xt[:, sl],
                                 start=True, stop=True)
                nc.tensor.matmul(ps_nh[:, 1, :], Whh_e[:, 2 * H:3 * H], hb_old[:],
                                 start=True, stop=False)
                nc.tensor.matmul(ps_nh[:, 1, :], BR[0:1, 0:H], ones[:, sl],
                                 start=False, stop=True)
                hb_new = gru_tail(ps_rz, ps_nh, bre, bze, bine, hb_old, hf)

                # Uo[:, t, sl] = U @ h_new + Ub   (bf16)
                ps_uo = ppw.tile([H, HB], F32, tag="wh")
                nc.tensor.matmul(ps_uo[:], UW[:], hb_new[:], start=True, stop=True)
                nc.scalar.activation(Uo[:, t, sl], ps_uo[:], AF.Identity, bias=Ub)

                # encb[:, 2hf+c2, :, t] = h_new.T chunks (bf16)
                for c2 in range(NC2):
                    ptr = pptr.tile([128, 128], BF16, tag="trb")
                    nc.tensor.transpose(ptr[0:128, 0:H],
                                        hb_new[:, c2 * 128:(c2 + 1) * 128],
                                        idb[0:H, 0:H])
                    nc.scalar.copy(encb[:, NC2 * hf + c2, :, t], ptr[0:128, 0:H])
                hbb[hf] = hb_new

        # ---------------- phase C: decoder ----------------
        prevs = []
        for hf in range(NH):
            pv = hp.tile([1, HB], BF16, tag=f"pv{hf}")
            nc.sync.dma_start(out=pv[:], in_=enc_d[T - 1, 0:1, hf * HB:(hf + 1) * HB])
            prevs.append(pv)

        for s in range(ks):
            dxt = sp.tile([DEC, BS], BF16, tag="dx")
            nc.sync.dma_start(out=dxt[:], in_=dec_d[s])

            scfs, attns = {}, {}
            # stage W + scores pipeline (t-chunks of TQ) per half
            for hf in range(NH):
                sl = slice(hf * HB, (hf + 1) * HB)
                ps_wh = ppw.tile([H, HB], F32, tag="wh")
                nc.tensor.matmul(ps_wh[:], WlW[:], hbb[hf][:], start=True, stop=True)
                wh = mp.tile([H, 1, HB], BF16, tag="whb")
                nc.scalar.activation(wh[:, 0, :], ps_wh[:], AF.Identity, bias=Wlb)
                scf = sp.tile([128, NC2, T], F32, tag="sco")
                for q in range(T // TQ):
                    tq = slice(q * TQ, (q + 1) * TQ)
                    xq = xqp.tile([H, TQ, HB], BF16, tag="xt")
                    nc.vector.tensor_add(xq[:], Uo[:, tq, sl],
                                         wh[:].broadcast_to((H, TQ, HB)))
                    nc.scalar.activation(xq[:], xq[:], AF.Tanh)
                    # rhs iterates t-outer, b-inner so the moving stream reads
                    # contiguous 32-col bursts from xq's [t, b] free layout
                    for c2 in range(NC2):
                        pssc = ppsc.tile([128, TQ, BBLK], F32, tag="sc")
                        for j in range(4):
                            b0 = c2 * 128 + j * BBLK
                            nc.tensor.matmul(pssc[BBLK * j:BBLK * j + 1, :, :], Vb[:],
                                             xq[:, :, b0:b0 + BBLK],
                                             start=True, stop=True,
                                             tile_position=(0, BBLK * j))
                        sstg = mp.tile([128, BBLK, TQ], F32, tag="sst")
                        nc.scalar.copy(sstg[:], pssc[:].transpose([0, 2, 1]))
                        nc.sync.dma_start(out=scf[:, c2, tq],
                                          in_=sstg[0:128:BBLK])
                scfs[hf] = scf

            # stage softmax (no max-subtraction; scores bounded by ||V||_1)
            # + attention weighted sum + transpose back
            for hf in range(NH):
                scf = scfs[hf]
                expo = sp.tile([128, NC2, T], F32, tag="expo")
                nc.scalar.activation(expo[:], scf[:], AF.Exp)
                sm = sp.tile([128, NC2], F32, tag="red")
                nc.vector.tensor_reduce(sm[:], expo[:], axis=AX.X, op=ALU.add)
                inv = sp.tile([128, NC2], F32, tag="red2")
                nc.vector.reciprocal(inv[:], sm[:])
                ab = sp.tile([128, NC2, 1, T], BF16, tag="ab")
                for c2 in range(NC2):
                    nc.vector.tensor_scalar_mul(ab[:, c2, 0, :], expo[:, c2, :],
                                                inv[:, c2:c2 + 1])
                attn_h = mp.tile([H, HB], BF16, tag="ah")
                TH = T // 2
                for c2 in range(NC2):
                    # alpha-weighted products, then in-place bf16 fold tree
                    # (TT adds run 2x; TensorReduce only 1x) down to 7 t-slots
                    P1 = pbp.tile([128, H, TH], BF16, tag="P")
                    nc.vector.tensor_mul(
                        P1[:], encb[:, NC2 * hf + c2, :, 0:TH],
                        ab[:, c2, :, 0:TH].broadcast_to((128, H, TH)))
                    P2 = pbp.tile([128, H, TH], BF16, tag="P")
                    nc.vector.tensor_mul(
                        P2[:], encb[:, NC2 * hf + c2, :, TH:T],
                        ab[:, c2, :, TH:T].broadcast_to((128, H, TH)))
                    nc.vector.tensor_add(P1[:], P1[:], P2[:])
                    nc.vector.tensor_add(P1[:, :, 0:14], P1[:, :, 0:14], P1[:, :, 14:28])
                    nc.vector.tensor_add(P1[:, :, 0:7], P1[:, :, 0:7], P1[:, :, 7:14])
                    attnc = sp.tile([128, H], BF16, tag="attnc")
                    with nc.allow_low_precision(reason="bf16 attn t-reduce"):
                        nc.vector.tensor_reduce(attnc[:], P1[:, :, 0:7], axis=AX.X,
                                                op=ALU.add)
                    ptr = pptr.tile([128, 128], BF16, tag="trb")
                    nc.tensor.transpose(ptr[0:H, 0:128], attnc[:], idb[:])
                    nc.scalar.copy(attn_h[:, c2 * 128:(c2 + 1) * 128], ptr[0:H, 0:128])
                attns[hf] = attn_h

            # stage gates + tail + out per half
            for hf in range(NH):
                sl = slice(hf * HB, (hf + 1) * HB)
                hb_old = hbb[hf]
                attn_h = attns[hf]
                prev = prevs[hf]
                ps_rz = ppg.tile([H, 2, HB], F32, tag="grz")
                for gi, g0 in ((0, 0), (1, H)):
                    nc.tensor.matmul(ps_rz[:, gi, :], WihP[:, g0:g0 + H], prev[:],
                                     start=True, stop=False)
                    nc.tensor.matmul(ps_rz[:, gi, :], WihX[:, g0:g0 + H], dxt[:, sl],
                                     start=False, stop=False)
                    nc.tensor.matmul(ps_rz[:, gi, :], WihA[:, g0:g0 + H], attn_h[:],
                                     start=False, stop=False)
                    nc.tensor.matmul(ps_rz[:, gi, :], Whh_dd[:, g0:g0 + H], hb_old[:],
                                     start=False, stop=True)
                ps_nh = ppg.tile([H, 2, HB], F32, tag="gnh")
                g0 = 2 * H
                nc.tensor.matmul(ps_nh[:, 0, :], WihP[:, g0:g0 + H], prev[:],
                                 start=True, stop=False)
                nc.tensor.matmul(ps_nh[:, 0, :], WihX[:, g0:g0 + H], dxt[:, sl],
                                 start=False, stop=False)
                nc.tensor.matmul(ps_nh[:, 0, :], WihA[:, g0:g0 + H], attn_h[:],
                                 start=False, stop=True)
                nc.tensor.matmul(ps_nh[:, 1, :], Whh_dd[:, g0:g0 + H], hb_old[:],
                                 start=True, stop=False)
                nc.tensor.matmul(ps_nh[:, 1, :], BR[0:1, H:2 * H], ones[:, sl],
                                 start=False, stop=True)
                hb_new = gru_tail(ps_rz, ps_nh, brd, bzd, bind, hb_old, hf)

                # out_s = h2o @ h_new + b  -> DRAM, also feeds prev
                ps_o = pptr.tile([1, HB], F32, tag="osc")
                nc.tensor.matmul(ps_o[:], h2oWb[:], hb_new[:], start=True, stop=True)
                pv = hp.tile([1, HB], BF16, tag=f"pv{hf}")
                nc.scalar.activation(pv[:], ps_o[:], AF.Identity, bias=h2ob[:, 0:1])
                nc.sync.dma_start(out=out_d[s, sl], in_=pv[:])
                prevs[hf] = pv
                hbb[hf] = hb_new
    _split_excess_waits(nc)
    return nc


def _host_inputs(inputs):
    import ml_dtypes
    f = lambda a: np.ascontiguousarray(a, dtype=np.float32)
    g = lambda a: np.ascontiguousarray(np.asarray(a, dtype=np.float32),
                                       dtype=ml_dtypes.bfloat16)
    Whh_e = np.asarray(inputs["enc_Whh"]).T.copy()   # [H, 3H]
    Whh_d = np.asarray(inputs["dec_Whh"]).T.copy()
    Whh_e[:, 2 * H:3 * H] *= 0.5
    Whh_d[:, 2 * H:3 * H] *= 0.5
    shared = {
        "W1": g(inputs["s2h_W1"].T), "W2": g(inputs["s2h_W2"].T),
        "b1": f(np.asarray(inputs["s2h_b1"]).reshape(96, 1)),
        "Wih_e": g(inputs["enc_Wih"].T), "Whh_e": g(Whh_e),
        "WihP_d": g(inputs["dec_Wih"][:, 0:1].T),
        "WihX_d": g(inputs["dec_Wih"][:, 1:1 + DEC].T),
        "WihA_d": g(inputs["dec_Wih"][:, 1 + DEC:].T),
        "Whh_d": g(Whh_d),
        "UW": g(inputs["U_W"].T), "WlW": g(inputs["Wl_W"].T),
        "h2ob": f(np.asarray(inputs["h2o_b"]).reshape(1, 1)),
        "ident": f(np.eye(128)),
    }
    BI = np.zeros((H, 16), dtype=np.float32)
    ebih, ebhh = np.asarray(inputs["enc_bih"]), np.asarray(inputs["enc_bhh"])
    dbih, dbhh = np.asarray(inputs["dec_bih"]), np.asarray(inputs["dec_bhh"])
    BI[:, 0] = (ebih[0:H] + ebhh[0:H]) / 2
    BI[:, 1] = (ebih[H:2 * H] + ebhh[H:2 * H]) / 2
    BI[:, 2] = ebih[2 * H:3 * H]
    BI[:, 4] = (dbih[0:H] + dbhh[0:H]) / 2
    BI[:, 5] = (dbih[H:2 * H] + dbhh[H:2 * H]) / 2
    BI[:, 6] = dbih[2 * H:3 * H]
    BI[:, 8] = np.asarray(inputs["U_b"])
    BI[:, 9] = np.asarray(inputs["Wl_b"])
    BI[:, 10] = np.asarray(inputs["V_W"]).reshape(-1)
    BI[:, 11] = np.asarray(inputs["h2o_W"]).reshape(-1)
    BI[:, 12] = np.asarray(inputs["s2h_b2"])
    shared["BIAS"] = f(BI)
    BR = np.zeros((1, 2 * H), dtype=np.float32)
    BR[0, 0:H] = ebhh[2 * H:3 * H] / 2
    BR[0, H:2 * H] = dbhh[2 * H:3 * H] / 2
    shared["BR"] = g(BR)

    enc = np.asarray(inputs["encoder_data"])   # [T, B, ENC]
    dec = np.asarray(inputs["decoder_data"])   # [S, B, DEC]
    ann = np.asarray(inputs["ann_data"])       # [B, ANN]
    maps = []
    for i in range(NCORES):
        sl = slice(i * BS, (i + 1) * BS)
        m = dict(shared)
        m["enc"] = g(enc[:, sl, :].transpose(0, 2, 1))
        m["dec"] = g(dec[:, sl, :].transpose(0, 2, 1))
        m["ann"] = g(ann[sl, :].T)
        maps.append(m)
    return maps


def kernel(**inputs) -> np.ndarray:
    from concourse.bass_utils import run_bass_kernel_spmd
    if "nc" not in _CACHE:
        _CACHE["nc"] = _build_program()
    nc = _CACHE["nc"]
    maps = _host_inputs(inputs)
    import os
    kw = {}
    if os.environ.get("KERNEL_TRACE") == "1":
        kw = dict(trace=True, trace_cores=[0])
    res = run_bass_kernel_spmd(nc, maps, list(range(NCORES)), **kw)
    _CACHE["last_res"] = res
    outs = [np.asarray(res.results[i]["out"], dtype=np.float32) for i in range(NCORES)]
    full = np.stack(outs, axis=1)              # [S, 8, 512]
    return full.reshape(S, B, 1)


if __name__ == "__main__":
    rng = np.random.default_rng(0)
    fake = {}
    fake["ann_data"] = rng.standard_normal((B, ANN), dtype=np.float32)
    fake["encoder_data"] = rng.standard_normal((T, B, ENC), dtype=np.float32)
    fake["decoder_data"] = rng.standard_normal((S, B, DEC), dtype=np.float32)
    p = lambda *s: (rng.standard_normal(s, dtype=np.float32) * 0.08)
    fake.update({
        "s2h_W1": p(96, ANN), "s2h_b1": p(96), "s2h_W2": p(H, 96), "s2h_b2": p(H),
        "enc_Wih": p(3 * H, ENC), "enc_Whh": p(3 * H, H),
        "enc_bih": p(3 * H), "enc_bhh": p(3 * H),
        "dec_Wih": p(3 * H, 1 + DEC + H), "dec_Whh": p(3 * H, H),
        "dec_bih": p(3 * H), "dec_bhh": p(3 * H),
        "U_W": p(H, H), "U_b": p(H), "Wl_W": p(H, H), "Wl_b": p(H),
        "V_W": p(1, H), "V_b": p(1), "h2o_W": p(1, H), "h2o_b": p(1),
    })
    out = kernel(**fake)
    print("out", out.shape, out.dtype, float(np.abs(out).mean()))
